# revision 1
# baseline (speedup 1.0000x reference)
"""Trainium2 Bass kernel for nn_MultiDiscretePolicy.

Math:
  h   = relu(s @ W1 + b1)                         [B, 1024]
  aw  = h @ W2 + b2                               [B, 256]
  d   = aw @ Wd + db    (Wd = head_W[...,0]-head_W[...,1] transposed)
  out pair h: even = 1.0 if (logit0+g0) >= (logit1+g1) else 0.0, odd = 1-even
The reference's y + stop_grad(y_hard - y) is exactly one-hot in fp32, and
argmax(softmax(z)) == argmax(z), so the output reduces to the sign test
  even = (d >= gdn),  gdn = q0 - q1,  q_k = log(-log(u_k + EPS) + EPS)
(two Ln passes on ACT; all-Ln keeps a single activation table set loaded).

Sharding: pure data parallel over the batch dim across 8 cores.  Matmuls keep
features on PSUM partitions / batch on the moving free dim, so the only
transpose needed (s^T) is done on host.  f32r matmuls (12-bit mantissa
products, fp32 accumulate) at full PE rate.
"""
from contextlib import ExitStack

import numpy as np

import concourse.bass as bass
import concourse.mybir as mybir
import concourse.tile as tile
from concourse import bacc
from concourse import bass_utils
from concourse.bass import ts, ds

N_CORES = 8
B, S_DIM, H_DIM, A_DIM = 32768, 1024, 1024, 512
D_HEAD = A_DIM // 2
EPS = 1e-20
BB = 512           # batch columns per block (one PSUM bank of fp32)
f32 = mybir.dt.float32
f32r = mybir.dt.float32r
AFT = mybir.ActivationFunctionType
OP = mybir.AluOpType

LAST_EXEC_NS = None

_cache: dict = {}


def _build(rpc: int, has_db: bool, loop_iters: int | None = None):
    """Build the per-core kernel for `rpc` batch rows per core.

    loop_iters: timing-only mode — repeat the whole pass that many times
    inside a hardware For_i loop (same data each iteration).
    """
    nb = rpc // BB
    nc = bacc.Bacc("TRN2", target_bir_lowering=False, debug=False,
                   num_devices=N_CORES)

    sTd = nc.dram_tensor("sT", [S_DIM, rpc], f32r, kind="ExternalInput").ap()
    u2d = nc.dram_tensor("u2", [rpc, 2 * A_DIM], f32, kind="ExternalInput").ap()
    W1d = nc.dram_tensor("W1d", [S_DIM, H_DIM], f32r, kind="ExternalInput").ap()
    # W2/Wd arrive host-packed partition-major so one partition's data is a
    # single contiguous run (DMA chunk size drives HBM efficiency)
    W2d = nc.dram_tensor("W2d", [128, 8 * D_HEAD], f32r, kind="ExternalInput").ap()
    Wdd = nc.dram_tensor("Wdd", [128, 2 * A_DIM], f32r, kind="ExternalInput").ap()
    b1d = nc.dram_tensor("b1d", [128, 8], f32, kind="ExternalInput").ap()
    b2d = nc.dram_tensor("b2d", [128, 2], f32, kind="ExternalInput").ap()
    if has_db:
        dbd = nc.dram_tensor("dbd", [1, A_DIM], f32r, kind="ExternalInput").ap()
    # only the even elements of each output pair are shipped (odd = 1 - even),
    # as uint8 {0,1} — exact, since the fp32 output is exactly one-hot
    outd = nc.dram_tensor("out", [rpc, A_DIM], mybir.dt.uint8,
                          kind="ExternalOutput").ap()

    sTv = sTd.rearrange("(a p) b -> p a b", p=128)      # [128, 8, rpc]
    # u arrives host-permuted in row pairs: row = q*256 + p*2 + h, so each
    # partition line of a pair-load is 8KB contiguous
    u2v = u2d.rearrange("(q p h) m -> p q (h m)", p=128, h=2)
    # out leaves partition-major within each 2-block group: DRAM row
    # g*(128*x) + p*x + xx, so each group store writes x*512 contiguous
    # bytes per partition (host undoes the permutation)
    xgrp = 8 if nb % 2 == 0 else 4
    assert nb % 2 == 0 or nb == 1
    outv = outd.rearrange("(g p x) m -> p g x m", p=128, x=xgrp)

    with tile.TileContext(nc) as tc, ExitStack() as ctx:
        wp = ctx.enter_context(tc.tile_pool(name="weights", bufs=1))
        sT_pool = ctx.enter_context(tc.tile_pool(name="sTp", bufs=4))
        u_pool = ctx.enter_context(tc.tile_pool(name="up", bufs=4))
        p_pool = ctx.enter_context(tc.tile_pool(name="pp", bufs=2))
        q_pool = ctx.enter_context(tc.tile_pool(name="qp", bufs=2))
        gdn_pool = ctx.enter_context(tc.tile_pool(name="gdnp", bufs=6))
        hT_pool = ctx.enter_context(tc.tile_pool(name="hTp", bufs=10))
        awT_pool = ctx.enter_context(tc.tile_pool(name="awTp", bufs=3))
        out_pool = ctx.enter_context(tc.tile_pool(name="outp", bufs=3))
        # h and d tiles share one 6-slot tag so mm1 (block0's k-outer) and
        # mm3 time-share PSUM banks; awT holds the other 2 banks.
        hd_psp = ctx.enter_context(tc.tile_pool(name="hdps", bufs=6, space="PSUM"))
        a_psp = ctx.enter_context(tc.tile_pool(name="aps", bufs=2, space="PSUM"))

        W1v = W1d.rearrange("(a p) j -> p a j", p=128)
        if loop_iters is not None:
            loop_cm = tc.For_i(0, loop_iters, 1)
            loop_cm.__enter__()
        # sT is processed in 2-block groups so each DMA's partition line is
        # a 4KB contiguous run; group 0 is loaded chunk-by-chunk interleaved
        # with W1 so block0's k-outer matmuls track the DMA stream.
        ngrp = (nb + 1) // 2

        def g_cols(g):
            return min(2 * BB, rpc - g * 2 * BB)

        def sT_group_load(g, split):
            tiles = []
            for ka in range(2):
                sT_t = sT_pool.tile([128, 4, 2 * BB], f32r, name="sT_t")
                if not split:
                    nc.sync.dma_start(
                        sT_t[:, :, 0:g_cols(g)],
                        sTv[:, ka * 4:(ka + 1) * 4, ds(g * 2 * BB, g_cols(g))])
                tiles.append(sT_t)
            return tiles

        W1_sb = wp.tile([128, 8, H_DIM], f32r)
        g0_ts = sT_group_load(0, split=True)
        for k in range(8):
            if k == 0:
                nc.sync.dma_start(W1_sb[:, 0, 0:768], W1v[:, 0, 0:768])
            else:
                nc.sync.dma_start(W1_sb[:, k, :], W1v[:, k, :])
            if k == 0 and g_cols(0) > BB:
                # split block0's first chunk so the very first matmul is
                # gated on ~0.6MB of DMA instead of ~1MB
                nc.sync.dma_start(g0_ts[0][:, 0, 0:BB], sTv[:, 0, ds(0, BB)])
                nc.sync.dma_start(g0_ts[0][:, 0, BB:g_cols(0)],
                                  sTv[:, 0, ds(BB, g_cols(0) - BB)])
            else:
                nc.sync.dma_start(g0_ts[k // 4][:, k % 4, 0:g_cols(0)],
                                  sTv[:, k, ds(0, g_cols(0))])
            if k == 0:
                nc.sync.dma_start(W1_sb[:, 0, 768:1024], W1v[:, 0, 768:1024])
        b1_sb = wp.tile([128, 8], f32)
        nc.sync.dma_start(b1_sb[:], b1d[:])
        W2_sb = wp.tile([128, 8 * D_HEAD], f32r)
        nc.sync.dma_start(W2_sb[:], W2d[:])
        W2_v = W2_sb.rearrange("p (j d) -> p j d", j=8)
        Wd_sb = wp.tile([128, 2 * A_DIM], f32r)
        nc.sync.dma_start(Wd_sb[:], Wdd[:])
        Wd_v = Wd_sb.rearrange("p (a m) -> p a m", a=2)
        b2_sb = wp.tile([128, 2], f32)
        nc.sync.dma_start(b2_sb[:], b2d[:])
        eps_sb = wp.tile([128, 1], f32)
        nc.vector.memset(eps_sb[:], EPS)
        if has_db:
            db_sb = wp.tile([1, A_DIM], f32r)
            nc.sync.dma_start(db_sb[:], dbd[:])
            ones_sb = wp.tile([1, 128], f32r)
            nc.vector.memset(ones_sb[:].bitcast(f32), 1.0)

        cur_sT = g0_ts
        next_sT = None
        o_t = None

        for b0 in range(nb):
            g = b0 // 2
            bw = b0 % 2
            if bw == 0:
                o_t = out_pool.tile([128, 8, A_DIM], mybir.dt.uint8,
                                    name="o_t")
            else:
                # prefetch the NEXT 2-block sT group one block ahead — late
                # enough to keep the head DMA queue short, early enough that
                # the 4MB lands within one block period
                if g + 1 < ngrp:
                    next_sT = sT_group_load(g + 1, split=False)
            # this block's u row-pairs (each an 8KB-line 1MB load)
            up_ts = []
            for q in range(2):
                u_t = u_pool.tile([128, 2, 2 * A_DIM], f32, name="u_t")
                nc.sync.dma_start(
                    u_t[:].rearrange("p h m -> p (h m)"),
                    u2v[:, b0 * 2 + q, :])
                up_ts.append(u_t)
            u_ts = [up_ts[bs // 2][:, bs % 2, :] for bs in range(4)]

            def sT_at(k):
                return cur_sT[k // 4][:, k % 4, ds(bw * BB, BB)]

            # ---- gumbel: p = ln(u+eps); q = ln(-p+eps); gdn = q0-q1 ----
            # (for block 0 this is emitted after the matmuls: u arrives late
            # and the Lns must not block the relus in the ACT FIFO)
            gdn_ts = []
            q_ts = []

            def ln_part(bs):
                p_t = p_pool.tile([128, 2 * A_DIM], f32, name="p_t")
                nc.scalar.activation(p_t[:], u_ts[bs], AFT.Ln,
                                     bias=eps_sb[:], scale=1.0)
                q_t = q_pool.tile([128, 2 * A_DIM], f32, name="q_t")
                nc.scalar.activation(q_t[:], p_t[:], AFT.Ln,
                                     bias=eps_sb[:], scale=-1.0)
                q_ts.append(q_t)

            def gdn_part(bs):
                q_t = q_ts[bs]
                gdn_t = gdn_pool.tile([128, A_DIM], f32, name="gdn_t")
                nc.vector.tensor_tensor(gdn_t[:], q_t[:, 0::2], q_t[:, 1::2],
                                        OP.subtract)
                gdn_ts.append(gdn_t)

            def gumbel(bs):
                ln_part(bs)
                gdn_part(bs)

            if b0 > 0:
                # Lns go to the ACT queue now; the gdn DVE subs are emitted
                # late in the mm1 loop so they don't delay the relus there.
                for bs in range(4):
                    ln_part(bs)

            # ---- mm1 (+ mm2 interleaved one j-group behind for b0>0) ----
            hT_ts = []
            a_pss = [a_psp.tile([128, BB], f32, name="a_ps") for _ in range(2)]

            def mm2_partial(j):
                for dt_ in range(2):
                    nc.tensor.matmul(a_pss[dt_][:], W2_v[:, j, ts(dt_, 128)],
                                     hT_ts[j][:], start=(j == 0),
                                     stop=(j == 7), skip_group_check=True)

            if b0 == 0:
                # k-outer over j-groups of 6 then 2: the widest group keeps PE
                # ~90% busy while the W1/sT0 chunks stream in; first matmuls
                # need only W1 chunk 0 + the first sT quarter.
                for grp in (range(0, 6), range(6, 8)):
                    h_pss = [hd_psp.tile([128, BB], f32, name="h_ps",
                                         tag="ps") for _ in grp]
                    for k in range(8):
                        for gi, j in enumerate(grp):
                            nc.tensor.matmul(
                                h_pss[gi][:], W1_sb[:, k, ts(j, 128)],
                                sT_at(k), start=(k == 0), stop=(k == 7))
                    for gi, j in enumerate(grp):
                        hT_t = hT_pool.tile([128, BB], f32r, name="hT_t")
                        nc.vector.tensor_scalar(hT_t[:], h_pss[gi][:],
                                                b1_sb[:, j:j + 1], 0.0,
                                                OP.add, OP.max)
                        hT_ts.append(hT_t)
                for j in range(8):
                    mm2_partial(j)
            else:
                for j in range(8):
                    h_ps = hd_psp.tile([128, BB], f32, name="h_ps", tag="ps")
                    for k in range(8):
                        nc.tensor.matmul(h_ps[:], W1_sb[:, k, ts(j, 128)],
                                         sT_at(k), start=(k == 0),
                                         stop=(k == 7))
                    hT_t = hT_pool.tile([128, BB], f32r, name="hT_t")
                    nc.vector.tensor_scalar(hT_t[:], h_ps[:],
                                            b1_sb[:, j:j + 1], 0.0,
                                            OP.add, OP.max)
                    hT_ts.append(hT_t)
                    if 3 <= j <= 6:
                        gdn_part(j - 3)
                    if j >= 2:
                        mm2_partial(j - 2)
                mm2_partial(6)
                mm2_partial(7)

            awT_ts = []
            for dt_ in range(2):
                awT_t = awT_pool.tile([128, BB], f32r, name="awT_t")
                nc.vector.tensor_scalar_add(awT_t[:], a_pss[dt_][:],
                                            b2_sb[:, dt_:dt_ + 1])
                awT_ts.append(awT_t)

            if b0 == 0:
                for bs in range(4):
                    gumbel(bs)

            # ---- mm3 + compare/emit per 128-row group ----
            for bs in range(4):
                d_ps = hd_psp.tile([128, A_DIM], f32, name="d_ps", tag="ps")
                for dt_ in range(2):
                    nc.tensor.matmul(d_ps[:], awT_ts[dt_][:, ts(bs, 128)],
                                     Wd_v[:, dt_, :], start=(dt_ == 0),
                                     stop=(dt_ == 1 and not has_db))
                if has_db:
                    nc.tensor.matmul(d_ps[:], ones_sb[:], db_sb[:],
                                     start=False, stop=True)
                nc.vector.tensor_tensor(o_t[:, bw * 4 + bs, :], d_ps[:],
                                        gdn_ts[bs][:], OP.is_ge)
            last_of_group = (bw == 1) or (b0 == nb - 1)
            if last_of_group:
                nx = 4 * (bw + 1)
                if b0 == nb - 1:
                    # split the tail store so the last DMA starts earlier
                    nc.sync.dma_start(outv[:, g, 0:nx // 2, :],
                                      o_t[:, 0:nx // 2, :])
                    h2 = (nx // 2 + nx) // 2
                    nc.sync.dma_start(outv[:, g, nx // 2:h2, :],
                                      o_t[:, nx // 2:h2, :])
                    nc.sync.dma_start(outv[:, g, h2:nx, :],
                                      o_t[:, h2:nx, :])
                else:
                    nc.sync.dma_start(outv[:, g, 0:nx, :], o_t[:, 0:nx, :])
                cur_sT = next_sT

        if loop_iters is not None:
            loop_cm.__exit__(None, None, None)

    nc.compile()
    return nc


def kernel(s, u, W1, b1, W2, b2, head_W, head_b, _rpc=None):
    global LAST_EXEC_NS
    s = np.asarray(s, dtype=np.float32)
    u = np.asarray(u, dtype=np.float32)
    W1 = np.ascontiguousarray(np.asarray(W1, dtype=np.float32))
    W2 = np.ascontiguousarray(np.asarray(W2, dtype=np.float32))
    b1 = np.asarray(b1, dtype=np.float32)
    b2 = np.asarray(b2, dtype=np.float32)
    head_W = np.asarray(head_W, dtype=np.float32)
    head_b = np.asarray(head_b, dtype=np.float32)

    nrows = s.shape[0]
    rpc = _rpc if _rpc is not None else nrows // N_CORES
    assert nrows == rpc * N_CORES and rpc % BB == 0

    sT = np.ascontiguousarray(s.T)                      # [S_DIM, nrows]
    u2 = u.reshape(nrows, 2 * A_DIM)
    # permute u rows to (q, p, h) pair-major per core shard (done per core
    # below), pack W2/Wd partition-major
    Wd = np.ascontiguousarray((head_W[:, :, 0] - head_W[:, :, 1]).T)
    W2h = np.ascontiguousarray(
        W2.reshape(8, 128, D_HEAD).transpose(1, 0, 2)).reshape(128, 8 * D_HEAD)
    Wdh = np.ascontiguousarray(
        Wd.reshape(2, 128, A_DIM).transpose(1, 0, 2)).reshape(128, 2 * A_DIM)
    db = np.ascontiguousarray(head_b[:, 0] - head_b[:, 1]).reshape(1, A_DIM)
    has_db = bool(np.any(db))
    b1c = np.ascontiguousarray(b1.reshape(8, 128).T)
    b2c = np.ascontiguousarray(b2.reshape(2, 128).T)

    key = (rpc, has_db)
    if key not in _cache:
        _cache[key] = _build(rpc, has_db)
    nc = _cache[key]

    nq = rpc // 256
    in_maps = []
    for c in range(N_CORES):
        uc = u2[c * rpc:(c + 1) * rpc]
        up = np.ascontiguousarray(
            uc.reshape(nq, 2, 128, 2 * A_DIM).transpose(0, 2, 1, 3)
        ).reshape(rpc, 2 * A_DIM)
        m = {
            "sT": np.ascontiguousarray(sT[:, c * rpc:(c + 1) * rpc]),
            "u2": up,
            "W1d": W1, "W2d": W2h, "Wdd": Wdh, "b1d": b1c, "b2d": b2c,
        }
        if has_db:
            m["dbd"] = db
        in_maps.append(m)

    res = bass_utils.run_bass_kernel_spmd(nc, in_maps,
                                          core_ids=list(range(N_CORES)))
    LAST_EXEC_NS = res.exec_time_ns
    nb = rpc // BB
    xgrp = 8 if nb % 2 == 0 else 4
    shards = []
    for c in range(N_CORES):
        e = res.results[c]["out"]                        # [rpc, A_DIM] uint8
        # undo the (g, p, x) store permutation back to batch order
        e = e.reshape(rpc // (128 * xgrp), 128, xgrp, A_DIM)
        shards.append(e.transpose(0, 2, 1, 3).reshape(rpc, A_DIM))
    evens = np.concatenate(shards, axis=0)               # [nrows, A_DIM]
    out = np.empty((nrows, 2 * A_DIM), dtype=np.float32)
    ef = evens.astype(np.float32)
    out[:, 0::2] = ef
    out[:, 1::2] = 1.0 - ef
    return out



# revision 31
# speedup vs baseline: 1.1086x; 1.1086x over previous
"""Trainium2 Bass kernel for nn_MultiDiscretePolicy.

Math:
  h   = relu(s @ W1 + b1)                         [B, 1024]
  aw  = h @ W2 + b2                               [B, 256]
  d   = aw @ Wd + db    (Wd = head_W[...,0]-head_W[...,1] transposed)
  out pair h: even = 1.0 if (logit0+g0) >= (logit1+g1) else 0.0, odd = 1-even
The reference's y + stop_grad(y_hard - y) is exactly one-hot in fp32, and
argmax(softmax(z)) == argmax(z), so the output reduces to the sign test
  even = (d >= gdn),  gdn = q0 - q1,  q_k = log(-log(u_k + EPS) + EPS)
(two Ln passes on ACT; all-Ln keeps a single activation table set loaded).

Sharding: pure data parallel over the batch dim across 8 cores.  Matmuls keep
features on PSUM partitions / batch on the moving free dim, so the only
transpose needed (s^T) is done on host.  f32r matmuls (12-bit mantissa
products, fp32 accumulate) at full PE rate.
"""
from contextlib import ExitStack

import ml_dtypes
import numpy as np

import concourse.bass as bass
import concourse.mybir as mybir
import concourse.tile as tile
from concourse import bacc
from concourse import bass_utils
from concourse.bass import ts, ds

N_CORES = 8
B, S_DIM, H_DIM, A_DIM = 32768, 1024, 1024, 512
D_HEAD = A_DIM // 2
EPS = 1e-20
BB = 512           # batch columns per block (one PSUM bank of fp32)
f32 = mybir.dt.float32
f32r = mybir.dt.float32r
bf16 = mybir.dt.bfloat16
AFT = mybir.ActivationFunctionType
OP = mybir.AluOpType

LAST_EXEC_NS = None

_cache: dict = {}


def _build(rpc: int, has_db: bool, loop_iters: int | None = None):
    """Build the per-core kernel for `rpc` batch rows per core.

    loop_iters: timing-only mode — repeat the whole pass that many times
    inside a hardware For_i loop (same data each iteration).
    """
    nb = rpc // BB
    nc = bacc.Bacc("TRN2", target_bir_lowering=False, debug=False,
                   num_devices=N_CORES)

    # s / W1 arrive in bf16: mm1 runs at the same PE rate (1 cycle/row) but
    # the startup-critical DMA halves, which is what gates the first block
    sTd = nc.dram_tensor("sT", [S_DIM, rpc], bf16, kind="ExternalInput").ap()
    u2d = nc.dram_tensor("u2", [rpc, 2 * A_DIM], f32, kind="ExternalInput").ap()
    W1d = nc.dram_tensor("W1d", [S_DIM, H_DIM], bf16, kind="ExternalInput").ap()
    # W2/Wd arrive host-packed partition-major so one partition's data is a
    # single contiguous run (DMA chunk size drives HBM efficiency)
    W2d = nc.dram_tensor("W2d", [128, 8 * D_HEAD], f32r, kind="ExternalInput").ap()
    Wdd = nc.dram_tensor("Wdd", [128, 2 * A_DIM], f32r, kind="ExternalInput").ap()
    b1d = nc.dram_tensor("b1d", [128, 8], f32, kind="ExternalInput").ap()
    b2d = nc.dram_tensor("b2d", [128, 2], f32, kind="ExternalInput").ap()
    identd = nc.dram_tensor("identd", [128, 128], f32r,
                            kind="ExternalInput").ap()
    if has_db:
        dbd = nc.dram_tensor("dbd", [1, A_DIM], f32r, kind="ExternalInput").ap()
    # only the even elements of each output pair are shipped (odd = 1 - even),
    # as uint8 {0,1} — exact, since the fp32 output is exactly one-hot
    outd = nc.dram_tensor("out", [rpc, A_DIM], mybir.dt.uint8,
                          kind="ExternalOutput").ap()

    sTv = sTd.rearrange("(a p) b -> p a b", p=128)      # [128, 8, rpc]
    # u arrives host-permuted in row pairs: row = q*256 + p*2 + h, so each
    # partition line of a pair-load is 8KB contiguous
    u2v = u2d.rearrange("(q p h) m -> p q (h m)", p=128, h=2)
    # out leaves partition-major within each 2-block group: DRAM row
    # g*(128*x) + p*x + xx, so each group store writes x*512 contiguous
    # bytes per partition (host undoes the permutation)
    xgrp = 8 if nb % 2 == 0 else 4
    assert nb % 2 == 0 or nb == 1
    outv = outd.rearrange("(g p x) m -> p g x m", p=128, x=xgrp)

    with tile.TileContext(nc) as tc, ExitStack() as ctx:
        wp = ctx.enter_context(tc.tile_pool(name="weights", bufs=1))
        sT_pool = ctx.enter_context(tc.tile_pool(name="sTp", bufs=4))
        u_pool = ctx.enter_context(tc.tile_pool(name="up", bufs=4))
        p_pool = ctx.enter_context(tc.tile_pool(name="pp", bufs=2))
        q_pool = ctx.enter_context(tc.tile_pool(name="qp", bufs=2))
        gdn_pool = ctx.enter_context(tc.tile_pool(name="gdnp", bufs=10))
        hT_pool = ctx.enter_context(tc.tile_pool(name="hTp", bufs=10))
        awT_pool = ctx.enter_context(tc.tile_pool(name="awTp", bufs=5))
        out_pool = ctx.enter_context(tc.tile_pool(name="outp", bufs=3))
        # h and d tiles share one 6-slot tag so mm1 (block0's k-outer) and
        # mm3 time-share PSUM banks; awT holds the other 2 banks.
        hd_psp = ctx.enter_context(tc.tile_pool(name="hdps", bufs=6, space="PSUM"))
        a_psp = ctx.enter_context(tc.tile_pool(name="aps", bufs=2, space="PSUM"))

        W1v = W1d.rearrange("(a p) j -> p a j", p=128)
        if loop_iters is not None:
            loop_cm = tc.For_i(0, loop_iters, 1)
            loop_cm.__enter__()
        # sT is processed in 2-block groups so each DMA's partition line is
        # a 4KB contiguous run; group 0 is loaded chunk-by-chunk interleaved
        # with W1 so block0's k-outer matmuls track the DMA stream.
        ngrp = (nb + 1) // 2

        def g_cols(g):
            return min(2 * BB, rpc - g * 2 * BB)

        def sT_group_load(g, split):
            tiles = []
            for ka in range(2):
                sT_t = sT_pool.tile([128, 4, 2 * BB], bf16, name="sT_t")
                if not split:
                    nc.sync.dma_start(
                        sT_t[:, :, 0:g_cols(g)],
                        sTv[:, ka * 4:(ka + 1) * 4, ds(g * 2 * BB, g_cols(g))])
                tiles.append(sT_t)
            return tiles

        W1_sb = wp.tile([128, 8, H_DIM], bf16)
        g0_ts = sT_group_load(0, split=True)
        # stream block-0's needs first: per k, the W1 chunk + only block-0's
        # 512 sT columns (1092ns DMA vs 1278ns of 6-wide j-group PE work per
        # k, so the PE never starves); block-1's columns follow afterwards
        gc0 = g_cols(0)
        ca = min(BB, gc0)
        for k in range(8):
            if k == 0:
                # tiny first piece so the very first matmul is gated on
                # ~100KB of DMA
                nc.sync.dma_start(W1_sb[:, 0, 0:256], W1v[:, 0, 0:256])
                nc.sync.dma_start(g0_ts[0][:, 0, 0:ca], sTv[:, 0, ds(0, ca)])
                nc.sync.dma_start(W1_sb[:, 0, 256:1024], W1v[:, 0, 256:1024])
            else:
                nc.sync.dma_start(W1_sb[:, k, :], W1v[:, k, :])
                nc.sync.dma_start(g0_ts[k // 4][:, k % 4, 0:ca],
                                  sTv[:, k, ds(0, ca)])
        b1_sb = wp.tile([128, 8], f32)
        nc.sync.dma_start(b1_sb[:], b1d[:])
        b2_sb = wp.tile([128, 2], f32)
        nc.sync.dma_start(b2_sb[:], b2d[:])
        W2_sb = wp.tile([128, 8 * D_HEAD], f32r)
        # split so mm2's first j-chunks don't wait on the whole 1MB load
        nc.sync.dma_start(W2_sb[:, 0:2 * D_HEAD], W2d[:, 0:2 * D_HEAD])
        nc.sync.dma_start(W2_sb[:, 2 * D_HEAD:], W2d[:, 2 * D_HEAD:])
        W2_v = W2_sb.rearrange("p (j d) -> p j d", j=8)
        Wd_sb = wp.tile([128, 2 * A_DIM], f32r)
        Wd_v = Wd_sb.rearrange("p (a m) -> p a m", a=2)
        ident_sb = wp.tile([128, 128], f32r)
        # block-1's sT columns and Wd interleave by need time: block-1 mm1
        # starts ~15us, the deferred block-0 mm3 needs Wd just after
        if gc0 > BB:
            for k in range(4):
                nc.sync.dma_start(g0_ts[k // 4][:, k % 4, BB:gc0],
                                  sTv[:, k, ds(BB, gc0 - BB)])
        nc.sync.dma_start(Wd_sb[:], Wdd[:])
        nc.sync.dma_start(ident_sb[:], identd[:])
        if gc0 > BB:
            for k in range(4, 8):
                nc.sync.dma_start(g0_ts[k // 4][:, k % 4, BB:gc0],
                                  sTv[:, k, ds(BB, gc0 - BB)])
        eps_sb = wp.tile([128, 1], f32)
        nc.vector.memset(eps_sb[:], EPS)
        if has_db:
            db_sb = wp.tile([1, A_DIM], f32r)
            nc.sync.dma_start(db_sb[:], dbd[:])
            ones_sb = wp.tile([1, 128], f32r)
            nc.vector.memset(ones_sb[:].bitcast(f32), 1.0)

        cur_sT = g0_ts
        next_sT = None
        o_t = None
        # mm3/compare/store of block b are deferred into block b+1's mm1
        # (after its j==0 column) so the mm2->awT->mm3 seam never stalls PE
        pending = None

        def u_load(b):
            # u row-pairs for block b (each an 8KB-line 1MB load)
            up_ts = []
            for q in range(2):
                u_t = u_pool.tile([128, 2, 2 * A_DIM], f32, name="u_t")
                nc.sync.dma_start(
                    u_t[:].rearrange("p h m -> p (h m)"),
                    u2v[:, b * 2 + q, :])
                up_ts.append(u_t)
            return up_ts

        cur_up = u_load(0)
        next_up = None

        def emit_mm3(awTs, gdns, ot, g_, bw_, tail=False):
            if not tail:
                for bs in range(4):
                    d_ps = hd_psp.tile([128, A_DIM], f32, name="d_ps",
                                       tag="ps")
                    for dt_ in range(2):
                        nc.tensor.matmul(d_ps[:], awTs[dt_][:, ts(bs, 128)],
                                         Wd_v[:, dt_, :], start=(dt_ == 0),
                                         stop=(dt_ == 1 and not has_db))
                    if has_db:
                        nc.tensor.matmul(d_ps[:], ones_sb[:], db_sb[:],
                                         start=False, stop=True)
                    nc.vector.tensor_tensor(ot[:, bw_ * 4 + bs, :], d_ps[:],
                                            gdns[bs][:], OP.is_ge)
                    # store each 128-row slice as soon as it's compared: the
                    # kernel tail then only waits on one 64KB store
                    nc.sync.dma_start(outv[:, g_, bw_ * 4 + bs, :],
                                      ot[:, bw_ * 4 + bs, :])
                return
            # tail layout: odd slices fold -gdn into PSUM up front (no awT
            # dependency — these matmuls fill the mm2->awT seam), then all
            # dt0 matmuls (gated only on awT0), then all dt1. Odd slices
            # compare via Sign on ACT (halving the DVE chain) and store via
            # the Pool SWDGE path so descriptor-gen runs in parallel with
            # HWDGE's.
            d_pss = []
            for bs in range(4):
                d_ps = hd_psp.tile([128, A_DIM], f32, name="d_ps", tag="ps")
                d_pss.append(d_ps)
                if bs % 2 == 1:
                    nc.tensor.matmul(d_ps[:], ident_sb[:], gdns[bs][:],
                                     start=True, stop=False)
            for dt_ in range(2):
                # odd (Sign/Pool-store) slices first in the dt1 round so the
                # longest store chain starts earliest
                for bs in ((1, 3, 0, 2) if dt_ == 1 else range(4)):
                    folded = bs % 2 == 1
                    nc.tensor.matmul(d_pss[bs][:], awTs[dt_][:, ts(bs, 128)],
                                     Wd_v[:, dt_, :],
                                     start=(dt_ == 0 and not folded),
                                     stop=(dt_ == 1 and not has_db),
                                     skip_group_check=True)
                    if dt_ == 1 and has_db:
                        nc.tensor.matmul(d_pss[bs][:], ones_sb[:], db_sb[:],
                                         start=False, stop=True)
                    if dt_ == 1:
                        sl = ot[:, bw_ * 4 + bs, :]
                        if folded:
                            nc.scalar.activation(sl, d_pss[bs][:], AFT.Sign,
                                                 bias=0.0, scale=1.0)
                            nc.gpsimd.dma_start(outv[:, g_, bw_ * 4 + bs, :],
                                                sl)
                        else:
                            nc.vector.tensor_tensor(sl, d_pss[bs][:],
                                                    gdns[bs][:], OP.is_ge)
                            nc.sync.dma_start(outv[:, g_, bw_ * 4 + bs, :],
                                              sl)

        for b0 in range(nb):
            g = b0 // 2
            bw = b0 % 2
            if bw == 0:
                o_t = out_pool.tile([128, 8, A_DIM], mybir.dt.uint8,
                                    name="o_t")
            else:
                # prefetch the NEXT 2-block sT group one block ahead — late
                # enough to keep the head DMA queue short, early enough that
                # the 4MB lands within one block period
                if g + 1 < ngrp:
                    next_sT = sT_group_load(g + 1, split=False)
            # u is prefetched one block ahead so the Lns (and the awT adds
            # queued behind them on ACT) never head-of-line block on u DMA
            up_ts = cur_up
            if b0 + 1 < nb:
                next_up = u_load(b0 + 1)
            u_ts = [up_ts[bs // 2][:, bs % 2, :] for bs in range(4)]

            def sT_at(k):
                return cur_sT[k // 4][:, k % 4, ds(bw * BB, BB)]

            # ---- gumbel: p = ln(u+eps); q = ln(-p+eps); gdn = q0-q1 ----
            # (for block 0 this is emitted after the matmuls: u arrives late
            # and the Lns must not block the relus in the ACT FIFO)
            gdn_ts = []
            q_ts = []

            def ln_part(bs):
                p_t = p_pool.tile([128, 2 * A_DIM], f32, name="p_t")
                nc.scalar.activation(p_t[:], u_ts[bs], AFT.Ln,
                                     bias=eps_sb[:], scale=1.0)
                q_t = q_pool.tile([128, 2 * A_DIM], f32, name="q_t")
                nc.scalar.activation(q_t[:], p_t[:], AFT.Ln,
                                     bias=eps_sb[:], scale=-1.0)
                q_ts.append(q_t)

            def gdn_part(bs):
                q_t = q_ts[bs]
                if b0 == nb - 1 and bs % 2 == 1:
                    # last block, odd slices: negated gdn in f32r, to be
                    # folded into PSUM by an identity matmul so the compare
                    # can run as Sign on the otherwise-idle ACT engine
                    gdn_t = gdn_pool.tile([128, A_DIM], f32r, name="gdn_t",
                                          tag="gdnn", bufs=2)
                    nc.vector.tensor_tensor(gdn_t[:], q_t[:, 1::2],
                                            q_t[:, 0::2], OP.subtract)
                else:
                    gdn_t = gdn_pool.tile([128, A_DIM], f32, name="gdn_t")
                    nc.vector.tensor_tensor(gdn_t[:], q_t[:, 0::2],
                                            q_t[:, 1::2], OP.subtract)
                gdn_ts.append(gdn_t)

            def gumbel(bs):
                ln_part(bs)
                gdn_part(bs)

            if b0 > 0:
                # Lns go to the ACT queue now; the gdn DVE subs are emitted
                # late in the mm1 loop so they don't delay the relus there.
                for bs in range(4):
                    ln_part(bs)

            # ---- mm1 (+ mm2 interleaved one j-group behind for b0>0) ----
            hT_ts = []
            a_pss = [a_psp.tile([128, BB], f32, name="a_ps") for _ in range(2)]

            def mm2_partial(j):
                for dt_ in range(2):
                    nc.tensor.matmul(a_pss[dt_][:], W2_v[:, j, ts(dt_, 128)],
                                     hT_ts[j][:], start=(j == 0),
                                     stop=(j == 7), skip_group_check=True)

            if b0 == 0:
                # k-outer over j-groups of 6 then 2: the widest group keeps PE
                # ~90% busy while the W1/sT0 chunks stream in; first matmuls
                # need only W1 chunk 0 + the first sT quarter.
                for grp in (range(0, 6), range(6, 8)):
                    h_pss = [hd_psp.tile([128, BB], f32, name="h_ps",
                                         tag="ps") for _ in grp]
                    for k in range(8):
                        for gi, j in enumerate(grp):
                            nc.tensor.matmul(
                                h_pss[gi][:], W1_sb[:, k, ts(j, 128)],
                                sT_at(k), start=(k == 0), stop=(k == 7))
                    for gi, j in enumerate(grp):
                        hT_t = hT_pool.tile([128, BB], f32r, name="hT_t")
                        nc.vector.tensor_scalar(hT_t[:], h_pss[gi][:],
                                                b1_sb[:, j:j + 1], 0.0,
                                                OP.add, OP.max)
                        hT_ts.append(hT_t)
                for j in range(8):
                    mm2_partial(j)
            else:
                for j in range(8):
                    h_ps = hd_psp.tile([128, BB], f32, name="h_ps", tag="ps")
                    for k in range(8):
                        nc.tensor.matmul(h_ps[:], W1_sb[:, k, ts(j, 128)],
                                         sT_at(k), start=(k == 0),
                                         stop=(k == 7))
                    hT_t = hT_pool.tile([128, BB], f32r, name="hT_t")
                    nc.vector.tensor_scalar(hT_t[:], h_ps[:],
                                            b1_sb[:, j:j + 1], 0.0,
                                            OP.add, OP.max)
                    hT_ts.append(hT_t)
                    if j == 0 and pending is not None:
                        emit_mm3(*pending)
                        pending = None
                    if 3 <= j <= 6:
                        gdn_part(j - 3)
                    if j >= 2:
                        mm2_partial(j - 2)
                mm2_partial(6)
                mm2_partial(7)

            last = b0 == nb - 1
            awT_ts = []
            for dt_ in range(2):
                awT_t = awT_pool.tile([128, BB], f32r, name="awT_t")
                if b0 <= 2 or (last and dt_ == 1):
                    # blocks 0-2: the ACT Ln pipeline is still draining the
                    # startup u backlog, so awT would head-of-line block
                    # behind u-gated Lns there; last block: split engines so
                    # both mm3 operands are ready ~one op after mm2
                    nc.vector.tensor_scalar_add(awT_t[:], a_pss[dt_][:],
                                                b2_sb[:, dt_:dt_ + 1])
                else:
                    # ACT can read PSUM and Identity shares the Ln table set,
                    # so these adds cost no DVE time and no table reload
                    nc.scalar.activation(awT_t[:], a_pss[dt_][:],
                                         AFT.Identity,
                                         bias=b2_sb[:, dt_:dt_ + 1],
                                         scale=1.0)
                awT_ts.append(awT_t)

            if b0 == 0:
                for bs in range(4):
                    gumbel(bs)

            if last:
                if pending is not None:
                    emit_mm3(*pending)
                    pending = None
                emit_mm3(awT_ts, gdn_ts, o_t, g, bw, tail=True)
            else:
                pending = (awT_ts, gdn_ts, o_t, g, bw)
            cur_up = next_up
            if (bw == 1) or last:
                cur_sT = next_sT

        if loop_iters is not None:
            loop_cm.__exit__(None, None, None)

    nc.compile()
    return nc


def kernel(s, u, W1, b1, W2, b2, head_W, head_b, _rpc=None):
    global LAST_EXEC_NS
    s = np.asarray(s, dtype=np.float32)
    u = np.asarray(u, dtype=np.float32)
    W1 = np.ascontiguousarray(np.asarray(W1, dtype=np.float32))
    W2 = np.ascontiguousarray(np.asarray(W2, dtype=np.float32))
    b1 = np.asarray(b1, dtype=np.float32)
    b2 = np.asarray(b2, dtype=np.float32)
    head_W = np.asarray(head_W, dtype=np.float32)
    head_b = np.asarray(head_b, dtype=np.float32)

    nrows = s.shape[0]
    rpc = _rpc if _rpc is not None else nrows // N_CORES
    assert nrows == rpc * N_CORES and rpc % BB == 0

    sT = s.T.astype(ml_dtypes.bfloat16)                 # [S_DIM, nrows]
    u2 = u.reshape(nrows, 2 * A_DIM)
    # permute u rows to (q, p, h) pair-major per core shard (done per core
    # below), pack W2/Wd partition-major
    Wd = np.ascontiguousarray((head_W[:, :, 0] - head_W[:, :, 1]).T)
    W2h = np.ascontiguousarray(
        W2.reshape(8, 128, D_HEAD).transpose(1, 0, 2)).reshape(128, 8 * D_HEAD)
    Wdh = np.ascontiguousarray(
        Wd.reshape(2, 128, A_DIM).transpose(1, 0, 2)).reshape(128, 2 * A_DIM)
    db = np.ascontiguousarray(head_b[:, 0] - head_b[:, 1]).reshape(1, A_DIM)
    has_db = bool(np.any(db))
    b1c = np.ascontiguousarray(b1.reshape(8, 128).T)
    b2c = np.ascontiguousarray(b2.reshape(2, 128).T)

    key = (rpc, has_db)
    if key not in _cache:
        _cache[key] = _build(rpc, has_db)
    nc = _cache[key]

    nq = rpc // 256
    in_maps = []
    for c in range(N_CORES):
        uc = u2[c * rpc:(c + 1) * rpc]
        up = np.ascontiguousarray(
            uc.reshape(nq, 2, 128, 2 * A_DIM).transpose(0, 2, 1, 3)
        ).reshape(rpc, 2 * A_DIM)
        m = {
            "sT": np.ascontiguousarray(sT[:, c * rpc:(c + 1) * rpc]),
            "u2": up,
            "W1d": W1.astype(ml_dtypes.bfloat16),
            "W2d": W2h, "Wdd": Wdh, "b1d": b1c, "b2d": b2c,
            "identd": np.eye(128, dtype=np.float32),
        }
        if has_db:
            m["dbd"] = db
        in_maps.append(m)

    res = bass_utils.run_bass_kernel_spmd(nc, in_maps,
                                          core_ids=list(range(N_CORES)))
    LAST_EXEC_NS = res.exec_time_ns
    nb = rpc // BB
    xgrp = 8 if nb % 2 == 0 else 4
    shards = []
    for c in range(N_CORES):
        e = res.results[c]["out"]                        # [rpc, A_DIM] uint8
        # undo the (g, p, x) store permutation back to batch order
        e = e.reshape(rpc // (128 * xgrp), 128, xgrp, A_DIM)
        shards.append(e.transpose(0, 2, 1, 3).reshape(rpc, A_DIM))
    evens = np.concatenate(shards, axis=0)               # [nrows, A_DIM]
    out = np.empty((nrows, 2 * A_DIM), dtype=np.float32)
    ef = evens.astype(np.float32)
    out[:, 0::2] = ef
    out[:, 1::2] = 1.0 - ef
    return out



# revision 37
# speedup vs baseline: 1.1113x; 1.0024x over previous
"""Trainium2 Bass kernel for nn_MultiDiscretePolicy.

Math:
  h   = relu(s @ W1 + b1)                         [B, 1024]
  aw  = h @ W2 + b2                               [B, 256]
  d   = aw @ Wd + db    (Wd = head_W[...,0]-head_W[...,1] transposed)
  out pair h: even = 1.0 if (logit0+g0) >= (logit1+g1) else 0.0, odd = 1-even
The reference's y + stop_grad(y_hard - y) is exactly one-hot in fp32, and
argmax(softmax(z)) == argmax(z), so the output reduces to the sign test
  even = (d >= gdn),  gdn = q0 - q1,  q_k = log(-log(u_k + EPS) + EPS)
(two Ln passes on ACT; all-Ln keeps a single activation table set loaded).

Sharding: pure data parallel over the batch dim across 8 cores.  Matmuls keep
features on PSUM partitions / batch on the moving free dim, so the only
transpose needed (s^T) is done on host.  f32r matmuls (12-bit mantissa
products, fp32 accumulate) at full PE rate.
"""
from contextlib import ExitStack

import ml_dtypes
import numpy as np

import concourse.bass as bass
import concourse.mybir as mybir
import concourse.tile as tile
from concourse import bacc
from concourse import bass_utils
from concourse.bass import ts, ds

N_CORES = 8
B, S_DIM, H_DIM, A_DIM = 32768, 1024, 1024, 512
D_HEAD = A_DIM // 2
EPS = 1e-20
BB = 512           # batch columns per block (one PSUM bank of fp32)
f32 = mybir.dt.float32
f32r = mybir.dt.float32r
bf16 = mybir.dt.bfloat16
AFT = mybir.ActivationFunctionType
OP = mybir.AluOpType

LAST_EXEC_NS = None

_cache: dict = {}


def _build(rpc: int, has_db: bool, loop_iters: int | None = None):
    """Build the per-core kernel for `rpc` batch rows per core.

    loop_iters: timing-only mode — repeat the whole pass that many times
    inside a hardware For_i loop (same data each iteration).
    """
    nb = rpc // BB
    nc = bacc.Bacc("TRN2", target_bir_lowering=False, debug=False,
                   num_devices=N_CORES)

    # s / W1 arrive in bf16: mm1 runs at the same PE rate (1 cycle/row) but
    # the startup-critical DMA halves, which is what gates the first block
    sTd = nc.dram_tensor("sT", [S_DIM, rpc], bf16, kind="ExternalInput").ap()
    u2d = nc.dram_tensor("u2", [rpc, 2 * A_DIM], f32, kind="ExternalInput").ap()
    W1d = nc.dram_tensor("W1d", [S_DIM, H_DIM], bf16, kind="ExternalInput").ap()
    # W2/Wd arrive host-packed partition-major so one partition's data is a
    # single contiguous run (DMA chunk size drives HBM efficiency)
    W2d = nc.dram_tensor("W2d", [128, 8 * D_HEAD], f32r, kind="ExternalInput").ap()
    Wdd = nc.dram_tensor("Wdd", [128, 2 * A_DIM], f32r, kind="ExternalInput").ap()
    b1d = nc.dram_tensor("b1d", [128, 8], f32, kind="ExternalInput").ap()
    b2d = nc.dram_tensor("b2d", [128, 2], f32, kind="ExternalInput").ap()
    identd = nc.dram_tensor("identd", [128, 128], f32r,
                            kind="ExternalInput").ap()
    if has_db:
        dbd = nc.dram_tensor("dbd", [1, A_DIM], f32r, kind="ExternalInput").ap()
    # only the even elements of each output pair are shipped (odd = 1 - even),
    # as uint8 {0,1} — exact, since the fp32 output is exactly one-hot
    outd = nc.dram_tensor("out", [rpc, A_DIM], mybir.dt.uint8,
                          kind="ExternalOutput").ap()

    sTv = sTd.rearrange("(a p) b -> p a b", p=128)      # [128, 8, rpc]
    # u arrives host-permuted in row pairs: row = q*256 + p*2 + h, so each
    # partition line of a pair-load is 8KB contiguous
    u2v = u2d.rearrange("(q p h) m -> p q (h m)", p=128, h=2)
    # out leaves partition-major within each 2-block group: DRAM row
    # g*(128*x) + p*x + xx, so each group store writes x*512 contiguous
    # bytes per partition (host undoes the permutation)
    xgrp = 8 if nb % 2 == 0 else 4
    assert nb % 2 == 0 or nb == 1
    outv = outd.rearrange("(g p x) m -> p g x m", p=128, x=xgrp)

    with tile.TileContext(nc) as tc, ExitStack() as ctx:
        wp = ctx.enter_context(tc.tile_pool(name="weights", bufs=1))
        sT_pool = ctx.enter_context(tc.tile_pool(name="sTp", bufs=4))
        u_pool = ctx.enter_context(tc.tile_pool(name="up", bufs=4))
        p_pool = ctx.enter_context(tc.tile_pool(name="pp", bufs=2))
        q_pool = ctx.enter_context(tc.tile_pool(name="qp", bufs=2))
        gdn_pool = ctx.enter_context(tc.tile_pool(name="gdnp", bufs=10))
        hT_pool = ctx.enter_context(tc.tile_pool(name="hTp", bufs=10))
        awT_pool = ctx.enter_context(tc.tile_pool(name="awTp", bufs=5))
        out_pool = ctx.enter_context(tc.tile_pool(name="outp", bufs=3))
        # h and d tiles share one 6-slot tag so mm1 (block0's k-outer) and
        # mm3 time-share PSUM banks; awT holds the other 2 banks.
        hd_psp = ctx.enter_context(tc.tile_pool(name="hdps", bufs=6, space="PSUM"))
        a_psp = ctx.enter_context(tc.tile_pool(name="aps", bufs=2, space="PSUM"))

        W1v = W1d.rearrange("(a p) j -> p a j", p=128)
        if loop_iters is not None:
            loop_cm = tc.For_i(0, loop_iters, 1)
            loop_cm.__enter__()
        # sT is processed in 2-block groups so each DMA's partition line is
        # a 4KB contiguous run; group 0 is loaded chunk-by-chunk interleaved
        # with W1 so block0's k-outer matmuls track the DMA stream.
        ngrp = (nb + 1) // 2

        def g_cols(g):
            return min(2 * BB, rpc - g * 2 * BB)

        def sT_group_load(g, split):
            tiles = []
            for ka in range(2):
                sT_t = sT_pool.tile([128, 4, 2 * BB], bf16, name="sT_t")
                if not split:
                    nc.sync.dma_start(
                        sT_t[:, :, 0:g_cols(g)],
                        sTv[:, ka * 4:(ka + 1) * 4, ds(g * 2 * BB, g_cols(g))])
                tiles.append(sT_t)
            return tiles

        W1_sb = wp.tile([128, 8, H_DIM], bf16)
        g0_ts = sT_group_load(0, split=True)
        # stream block-0's needs first: per k, the W1 chunk + only block-0's
        # 512 sT columns (1092ns DMA vs 1278ns of 6-wide j-group PE work per
        # k, so the PE never starves); block-1's columns follow afterwards
        gc0 = g_cols(0)
        ca = min(BB, gc0)
        for k in range(8):
            if k == 0:
                # tiny first piece so the very first matmul is gated on
                # ~100KB of DMA
                nc.sync.dma_start(W1_sb[:, 0, 0:256], W1v[:, 0, 0:256])
                nc.sync.dma_start(g0_ts[0][:, 0, 0:ca], sTv[:, 0, ds(0, ca)])
                nc.sync.dma_start(W1_sb[:, 0, 256:1024], W1v[:, 0, 256:1024])
            else:
                nc.sync.dma_start(W1_sb[:, k, :], W1v[:, k, :])
                nc.sync.dma_start(g0_ts[k // 4][:, k % 4, 0:ca],
                                  sTv[:, k, ds(0, ca)])
        b1_sb = wp.tile([128, 8], f32)
        nc.sync.dma_start(b1_sb[:], b1d[:])
        b2_sb = wp.tile([128, 2], f32)
        nc.sync.dma_start(b2_sb[:], b2d[:])
        W2_sb = wp.tile([128, 8 * D_HEAD], f32r)
        # split so mm2's first j-chunks don't wait on the whole 1MB load
        nc.sync.dma_start(W2_sb[:, 0:2 * D_HEAD], W2d[:, 0:2 * D_HEAD])
        nc.sync.dma_start(W2_sb[:, 2 * D_HEAD:], W2d[:, 2 * D_HEAD:])
        W2_v = W2_sb.rearrange("p (j d) -> p j d", j=8)
        Wd_sb = wp.tile([128, 2 * A_DIM], f32r)
        Wd_v = Wd_sb.rearrange("p (a m) -> p a m", a=2)
        ident_sb = wp.tile([128, 128], f32r)
        # block-1's sT columns and Wd interleave by need time: block-1 mm1
        # starts ~15us, the deferred block-0 mm3 needs Wd just after
        if gc0 > BB:
            for k in range(4):
                nc.sync.dma_start(g0_ts[k // 4][:, k % 4, BB:gc0],
                                  sTv[:, k, ds(BB, gc0 - BB)])
        nc.sync.dma_start(Wd_sb[:], Wdd[:])
        nc.sync.dma_start(ident_sb[:], identd[:])
        if gc0 > BB:
            for k in range(4, 8):
                nc.sync.dma_start(g0_ts[k // 4][:, k % 4, BB:gc0],
                                  sTv[:, k, ds(BB, gc0 - BB)])
        eps_sb = wp.tile([128, 1], f32)
        nc.vector.memset(eps_sb[:], EPS)
        if has_db:
            db_sb = wp.tile([1, A_DIM], f32r)
            nc.sync.dma_start(db_sb[:], dbd[:])
            ones_sb = wp.tile([1, 128], f32r)
            nc.vector.memset(ones_sb[:].bitcast(f32), 1.0)

        cur_sT = g0_ts
        next_sT = None
        o_t = None
        # mm3/compare/store of block b are deferred into block b+1's mm1
        # (after its j==0 column) so the mm2->awT->mm3 seam never stalls PE
        pending = None

        def u_load(b):
            # u row-pairs for block b (each an 8KB-line 1MB load)
            up_ts = []
            for q in range(2):
                u_t = u_pool.tile([128, 2, 2 * A_DIM], f32, name="u_t")
                nc.sync.dma_start(
                    u_t[:].rearrange("p h m -> p (h m)"),
                    u2v[:, b * 2 + q, :])
                up_ts.append(u_t)
            return up_ts

        cur_up = u_load(0)
        next_up = None

        def emit_mm3(awTs, gdns, ot, g_, bw_, tail=False):
            if not tail:
                for bs in range(4):
                    d_ps = hd_psp.tile([128, A_DIM], f32, name="d_ps",
                                       tag="ps")
                    for dt_ in range(2):
                        nc.tensor.matmul(d_ps[:], awTs[dt_][:, ts(bs, 128)],
                                         Wd_v[:, dt_, :], start=(dt_ == 0),
                                         stop=(dt_ == 1 and not has_db))
                    if has_db:
                        nc.tensor.matmul(d_ps[:], ones_sb[:], db_sb[:],
                                         start=False, stop=True)
                    nc.vector.tensor_tensor(ot[:, bw_ * 4 + bs, :], d_ps[:],
                                            gdns[bs][:], OP.is_ge)
                    # store each 128-row slice as soon as it's compared: the
                    # kernel tail then only waits on one 64KB store
                    nc.sync.dma_start(outv[:, g_, bw_ * 4 + bs, :],
                                      ot[:, bw_ * 4 + bs, :])
                return
            # tail layout: odd slices fold -gdn into PSUM up front (no awT
            # dependency — these matmuls fill the mm2->awT seam), then all
            # dt0 matmuls (gated only on awT0), then all dt1. Odd slices
            # compare via Sign on ACT (halving the DVE chain) and store via
            # the Pool SWDGE path so descriptor-gen runs in parallel with
            # HWDGE's.
            d_pss = []
            for bs in range(4):
                d_ps = hd_psp.tile([128, A_DIM], f32, name="d_ps", tag="ps")
                d_pss.append(d_ps)
                nc.tensor.matmul(d_ps[:], ident_sb[:], gdns[bs][:],
                                 start=True, stop=False)
            for dt_ in range(2):
                # odd (Sign/Pool-store) slices first in the dt1 round so the
                # longest store chain starts earliest
                for bs in ((1, 3, 0, 2) if dt_ == 1 else range(4)):
                    nc.tensor.matmul(d_pss[bs][:], awTs[dt_][:, ts(bs, 128)],
                                     Wd_v[:, dt_, :], start=False,
                                     stop=(dt_ == 1 and not has_db),
                                     skip_group_check=True)
                    if dt_ == 1 and has_db:
                        nc.tensor.matmul(d_pss[bs][:], ones_sb[:], db_sb[:],
                                         start=False, stop=True)
                    if dt_ == 1:
                        sl = ot[:, bw_ * 4 + bs, :]
                        if bs % 2 == 1:
                            nc.scalar.activation(sl, d_pss[bs][:], AFT.Sign,
                                                 bias=0.0, scale=1.0)
                        else:
                            nc.vector.tensor_scalar(sl, d_pss[bs][:], 0.0,
                                                    None, OP.is_ge)
                        # queue split chosen so the 625ns-per-DMA HWDGE chain
                        # and the 1038ns Pool SWDGE gen finish together
                        if bs == 0:
                            nc.gpsimd.dma_start(outv[:, g_, bw_ * 4 + bs, :],
                                                sl)
                        else:
                            nc.sync.dma_start(outv[:, g_, bw_ * 4 + bs, :],
                                              sl)

        for b0 in range(nb):
            g = b0 // 2
            bw = b0 % 2
            if bw == 0:
                o_t = out_pool.tile([128, 8, A_DIM], mybir.dt.uint8,
                                    name="o_t")
            else:
                # prefetch the NEXT 2-block sT group one block ahead — late
                # enough to keep the head DMA queue short, early enough that
                # the 4MB lands within one block period
                if g + 1 < ngrp:
                    next_sT = sT_group_load(g + 1, split=False)
            # u is prefetched one block ahead so the Lns (and the awT adds
            # queued behind them on ACT) never head-of-line block on u DMA
            up_ts = cur_up
            if b0 + 1 < nb:
                next_up = u_load(b0 + 1)
            u_ts = [up_ts[bs // 2][:, bs % 2, :] for bs in range(4)]

            def sT_at(k):
                return cur_sT[k // 4][:, k % 4, ds(bw * BB, BB)]

            # ---- gumbel: p = ln(u+eps); q = ln(-p+eps); gdn = q0-q1 ----
            # (for block 0 this is emitted after the matmuls: u arrives late
            # and the Lns must not block the relus in the ACT FIFO)
            gdn_ts = []
            q_ts = []

            def ln_part(bs):
                p_t = p_pool.tile([128, 2 * A_DIM], f32, name="p_t")
                nc.scalar.activation(p_t[:], u_ts[bs], AFT.Ln,
                                     bias=eps_sb[:], scale=1.0)
                q_t = q_pool.tile([128, 2 * A_DIM], f32, name="q_t")
                nc.scalar.activation(q_t[:], p_t[:], AFT.Ln,
                                     bias=eps_sb[:], scale=-1.0)
                q_ts.append(q_t)

            def gdn_part(bs):
                q_t = q_ts[bs]
                if b0 == nb - 1:
                    # last block: negated gdn in f32r, folded into PSUM by an
                    # identity matmul (fills the mm2->awT seam with PE work);
                    # odd slices then compare via Sign on the idle ACT
                    # engine, even ones via an immediate-0 is_ge on DVE
                    gdn_t = gdn_pool.tile([128, A_DIM], f32r, name="gdn_t",
                                          tag="gdnn", bufs=4)
                    nc.vector.tensor_tensor(gdn_t[:], q_t[:, 1::2],
                                            q_t[:, 0::2], OP.subtract)
                else:
                    gdn_t = gdn_pool.tile([128, A_DIM], f32, name="gdn_t")
                    nc.vector.tensor_tensor(gdn_t[:], q_t[:, 0::2],
                                            q_t[:, 1::2], OP.subtract)
                gdn_ts.append(gdn_t)

            def gumbel(bs):
                ln_part(bs)
                gdn_part(bs)

            if b0 > 0:
                # Lns go to the ACT queue now; the gdn DVE subs are emitted
                # late in the mm1 loop so they don't delay the relus there.
                for bs in range(4):
                    ln_part(bs)

            # ---- mm1 (+ mm2 interleaved one j-group behind for b0>0) ----
            hT_ts = []
            a_pss = [a_psp.tile([128, BB], f32, name="a_ps") for _ in range(2)]

            def mm2_partial(j):
                for dt_ in range(2):
                    nc.tensor.matmul(a_pss[dt_][:], W2_v[:, j, ts(dt_, 128)],
                                     hT_ts[j][:], start=(j == 0),
                                     stop=(j == 7), skip_group_check=True)

            if b0 == 0:
                # k-outer over j-groups of 6 then 2: the widest group keeps PE
                # ~90% busy while the W1/sT0 chunks stream in; first matmuls
                # need only W1 chunk 0 + the first sT quarter.
                for grp in (range(0, 6), range(6, 8)):
                    h_pss = [hd_psp.tile([128, BB], f32, name="h_ps",
                                         tag="ps") for _ in grp]
                    for k in range(8):
                        for gi, j in enumerate(grp):
                            nc.tensor.matmul(
                                h_pss[gi][:], W1_sb[:, k, ts(j, 128)],
                                sT_at(k), start=(k == 0), stop=(k == 7))
                    for gi, j in enumerate(grp):
                        hT_t = hT_pool.tile([128, BB], f32r, name="hT_t")
                        nc.vector.tensor_scalar(hT_t[:], h_pss[gi][:],
                                                b1_sb[:, j:j + 1], 0.0,
                                                OP.add, OP.max)
                        hT_ts.append(hT_t)
                for j in range(8):
                    mm2_partial(j)
            else:
                last = b0 == nb - 1
                jmax = 7 if last else 8
                for j in range(jmax):
                    h_ps = hd_psp.tile([128, BB], f32, name="h_ps", tag="ps")
                    for k in range(8):
                        nc.tensor.matmul(h_ps[:], W1_sb[:, k, ts(j, 128)],
                                         sT_at(k), start=(k == 0),
                                         stop=(k == 7))
                    hT_t = hT_pool.tile([128, BB], f32r, name="hT_t")
                    nc.vector.tensor_scalar(hT_t[:], h_ps[:],
                                            b1_sb[:, j:j + 1], 0.0,
                                            OP.add, OP.max)
                    hT_ts.append(hT_t)
                    if j == 0 and pending is not None:
                        emit_mm3(*pending)
                        pending = None
                    if 3 <= j <= 6:
                        gdn_part(j - 3)
                    if j >= 2:
                        mm2_partial(j - 2)
                if not last:
                    mm2_partial(6)
                    mm2_partial(7)
                else:
                    # ---- last block: j7 + the whole epilogue run in two
                    # 256-column halves, so half 0's mm3/compare/store chain
                    # overlaps half 1's matmuls and the final compare chain
                    # is one op deep per engine instead of two ----
                    hT7 = hT_pool.tile([128, BB], f32r, name="hT_t")
                    for h in range(2):
                        h_ps = hd_psp.tile([128, 256], f32, name="h_ps",
                                           tag="ps")
                        for k in range(8):
                            nc.tensor.matmul(
                                h_ps[:], W1_sb[:, k, ts(7, 128)],
                                cur_sT[k // 4][:, k % 4,
                                               ds(bw * BB + h * 256, 256)],
                                start=(k == 0), stop=(k == 7))
                        nc.vector.tensor_scalar(hT7[:, h * 256:h * 256 + 256],
                                                h_ps[:], b1_sb[:, 7:8], 0.0,
                                                OP.add, OP.max)
                        mm2_partial(5 + h)
                    hT_ts.append(hT7)
                    for h in range(2):
                        sl2 = slice(h * 256, h * 256 + 256)
                        for dt_ in range(2):
                            nc.tensor.matmul(a_pss[dt_][:, sl2],
                                             W2_v[:, 7, ts(dt_, 128)],
                                             hT7[:, sl2], start=False,
                                             stop=True, skip_group_check=True)
                    dps = []
                    for bs in range(4):
                        d_ps = hd_psp.tile([128, A_DIM], f32,
                                           name="d_ps", tag="ps")
                        nc.tensor.matmul(d_ps[:], ident_sb[:],
                                         gdn_ts[bs][:], start=True,
                                         stop=False)
                        dps.append(d_ps)
                    awThs = []
                    for h in range(2):
                        sl2 = slice(h * 256, h * 256 + 256)
                        awTh = []
                        for dt_ in range(2):
                            awT_t = awT_pool.tile([128, 256], f32r,
                                                  name="awT_t", tag="awh",
                                                  bufs=4)
                            if dt_ == 1:
                                nc.vector.tensor_scalar_add(
                                    awT_t[:], a_pss[dt_][:, sl2],
                                    b2_sb[:, dt_:dt_ + 1])
                            else:
                                nc.scalar.activation(
                                    awT_t[:], a_pss[dt_][:, sl2],
                                    AFT.Identity,
                                    bias=b2_sb[:, dt_:dt_ + 1], scale=1.0)
                            awTh.append(awT_t)
                        awThs.append(awTh)
                    for h in range(2):
                        for dt_ in range(2):
                            for i_ in range(2):
                                bs = 2 * h + i_
                                nc.tensor.matmul(
                                    dps[bs][:], awThs[h][dt_][:, ts(i_, 128)],
                                    Wd_v[:, dt_, :], start=False,
                                    stop=(dt_ == 1 and not has_db),
                                    skip_group_check=True)
                                if dt_ == 1 and has_db:
                                    nc.tensor.matmul(dps[bs][:], ones_sb[:],
                                                     db_sb[:], start=False,
                                                     stop=True)
                        for i_ in range(2):
                            bs = 2 * h + i_
                            sl = o_t[:, bw * 4 + bs, :]
                            if i_ == 1:
                                nc.scalar.activation(sl, dps[bs][:], AFT.Sign,
                                                     bias=0.0, scale=1.0)
                            else:
                                nc.vector.tensor_scalar(sl, dps[bs][:], 0.0,
                                                        None, OP.is_ge)
                            # the very last slice takes the Pool SWDGE path
                            # (its gen runs while HWDGE drains the others)
                            if bs == 3:
                                nc.gpsimd.dma_start(
                                    outv[:, g, bw * 4 + bs, :], sl)
                            else:
                                nc.sync.dma_start(
                                    outv[:, g, bw * 4 + bs, :], sl)
                    cur_up = next_up
                    cur_sT = next_sT
                    continue

            last = b0 == nb - 1
            awT_ts = []
            for dt_ in range(2):
                awT_t = awT_pool.tile([128, BB], f32r, name="awT_t")
                if b0 <= 2 or (last and dt_ == 1):
                    # blocks 0-2: the ACT Ln pipeline is still draining the
                    # startup u backlog, so awT would head-of-line block
                    # behind u-gated Lns there; last block: split engines so
                    # both mm3 operands are ready ~one op after mm2
                    nc.vector.tensor_scalar_add(awT_t[:], a_pss[dt_][:],
                                                b2_sb[:, dt_:dt_ + 1])
                else:
                    # ACT can read PSUM and Identity shares the Ln table set,
                    # so these adds cost no DVE time and no table reload
                    nc.scalar.activation(awT_t[:], a_pss[dt_][:],
                                         AFT.Identity,
                                         bias=b2_sb[:, dt_:dt_ + 1],
                                         scale=1.0)
                awT_ts.append(awT_t)

            if b0 == 0:
                for bs in range(4):
                    gumbel(bs)

            if last:
                if pending is not None:
                    emit_mm3(*pending)
                    pending = None
                emit_mm3(awT_ts, gdn_ts, o_t, g, bw, tail=True)
            else:
                pending = (awT_ts, gdn_ts, o_t, g, bw)
            cur_up = next_up
            if (bw == 1) or last:
                cur_sT = next_sT

        if loop_iters is not None:
            loop_cm.__exit__(None, None, None)

    nc.compile()
    return nc


def kernel(s, u, W1, b1, W2, b2, head_W, head_b, _rpc=None):
    global LAST_EXEC_NS
    s = np.asarray(s, dtype=np.float32)
    u = np.asarray(u, dtype=np.float32)
    W1 = np.ascontiguousarray(np.asarray(W1, dtype=np.float32))
    W2 = np.ascontiguousarray(np.asarray(W2, dtype=np.float32))
    b1 = np.asarray(b1, dtype=np.float32)
    b2 = np.asarray(b2, dtype=np.float32)
    head_W = np.asarray(head_W, dtype=np.float32)
    head_b = np.asarray(head_b, dtype=np.float32)

    nrows = s.shape[0]
    rpc = _rpc if _rpc is not None else nrows // N_CORES
    assert nrows == rpc * N_CORES and rpc % BB == 0

    sT = s.T.astype(ml_dtypes.bfloat16)                 # [S_DIM, nrows]
    u2 = u.reshape(nrows, 2 * A_DIM)
    # permute u rows to (q, p, h) pair-major per core shard (done per core
    # below), pack W2/Wd partition-major
    Wd = np.ascontiguousarray((head_W[:, :, 0] - head_W[:, :, 1]).T)
    W2h = np.ascontiguousarray(
        W2.reshape(8, 128, D_HEAD).transpose(1, 0, 2)).reshape(128, 8 * D_HEAD)
    Wdh = np.ascontiguousarray(
        Wd.reshape(2, 128, A_DIM).transpose(1, 0, 2)).reshape(128, 2 * A_DIM)
    db = np.ascontiguousarray(head_b[:, 0] - head_b[:, 1]).reshape(1, A_DIM)
    has_db = bool(np.any(db))
    b1c = np.ascontiguousarray(b1.reshape(8, 128).T)
    b2c = np.ascontiguousarray(b2.reshape(2, 128).T)

    key = (rpc, has_db)
    if key not in _cache:
        _cache[key] = _build(rpc, has_db)
    nc = _cache[key]

    nq = rpc // 256
    in_maps = []
    for c in range(N_CORES):
        uc = u2[c * rpc:(c + 1) * rpc]
        up = np.ascontiguousarray(
            uc.reshape(nq, 2, 128, 2 * A_DIM).transpose(0, 2, 1, 3)
        ).reshape(rpc, 2 * A_DIM)
        m = {
            "sT": np.ascontiguousarray(sT[:, c * rpc:(c + 1) * rpc]),
            "u2": up,
            "W1d": W1.astype(ml_dtypes.bfloat16),
            "W2d": W2h, "Wdd": Wdh, "b1d": b1c, "b2d": b2c,
            "identd": np.eye(128, dtype=np.float32),
        }
        if has_db:
            m["dbd"] = db
        in_maps.append(m)

    res = bass_utils.run_bass_kernel_spmd(nc, in_maps,
                                          core_ids=list(range(N_CORES)))
    LAST_EXEC_NS = res.exec_time_ns
    nb = rpc // BB
    xgrp = 8 if nb % 2 == 0 else 4
    shards = []
    for c in range(N_CORES):
        e = res.results[c]["out"]                        # [rpc, A_DIM] uint8
        # undo the (g, p, x) store permutation back to batch order
        e = e.reshape(rpc // (128 * xgrp), 128, xgrp, A_DIM)
        shards.append(e.transpose(0, 2, 1, 3).reshape(rpc, A_DIM))
    evens = np.concatenate(shards, axis=0)               # [nrows, A_DIM]
    out = np.empty((nrows, 2 * A_DIM), dtype=np.float32)
    ef = evens.astype(np.float32)
    out[:, 0::2] = ef
    out[:, 1::2] = 1.0 - ef
    return out



# revision 49
# speedup vs baseline: 1.1170x; 1.0051x over previous
"""Trainium2 Bass kernel for nn_MultiDiscretePolicy.

Math:
  h   = relu(s @ W1 + b1)                         [B, 1024]
  aw  = h @ W2 + b2                               [B, 256]
  d   = aw @ Wd + db    (Wd = head_W[...,0]-head_W[...,1] transposed)
  out pair h: even = 1.0 if (logit0+g0) >= (logit1+g1) else 0.0, odd = 1-even
The reference's y + stop_grad(y_hard - y) is exactly one-hot in fp32, and
argmax(softmax(z)) == argmax(z), so the output reduces to the sign test
  even = (d >= gdn),  gdn = q0 - q1,  q_k = log(-log(u_k + EPS) + EPS)
(two Ln passes on ACT; Ln/Identity/Sign share one activation table set).

Sharding: pure data parallel over the batch dim across 8 cores.  Matmuls keep
features on PSUM partitions / batch on the moving free dim, so the only
transpose needed (s^T) is done on host.

The kernel is PE-flop-bound (~151us of matmul at 1 cycle/row), so the layout
is built around keeping the tensor engine saturated:
  - mm1 runs in bf16 (same PE rate as f32r, half the DMA): the startup
    W1+sT stream halves, which is what gated block 0.  Costs ~960 flipped
    sign-test pairs out of 16.8M (~1.1e-2 rel err, budget is 2e-2).
  - mm2/mm3 stay f32r (12-bit mantissa products, fp32 accumulate).
  - mm3/compare/store of block b are deferred into block b+1's mm1 so the
    mm2->awT(DVE/ACT)->mm3 seam never stalls PE; u is prefetched one block
    ahead so the ACT Ln queue never head-of-line blocks the awT adds.
  - the last block's j7/mm2/mm3 run in two 256-column halves, -gdn is folded
    into PSUM via an identity matmul, and the final compares run one-deep on
    DVE (is_ge 0) and ACT (Sign, which saturates to exact {0,1} in uint8),
    with the last store on the Pool SWDGE path: the post-matmul tail is
    ~3.6us (compare + store + DMA-sem + drain).
"""
from contextlib import ExitStack

import ml_dtypes
import numpy as np

import concourse.bass as bass
import concourse.mybir as mybir
import concourse.tile as tile
from concourse import bacc
from concourse import bass_utils
from concourse.bass import ts, ds

N_CORES = 8
B, S_DIM, H_DIM, A_DIM = 32768, 1024, 1024, 512
D_HEAD = A_DIM // 2
EPS = 1e-20
BB = 512           # batch columns per block (one PSUM bank of fp32)
f32 = mybir.dt.float32
f32r = mybir.dt.float32r
bf16 = mybir.dt.bfloat16
AFT = mybir.ActivationFunctionType
OP = mybir.AluOpType

LAST_EXEC_NS = None

_cache: dict = {}


def _build(rpc: int, has_db: bool, loop_iters: int | None = None):
    """Build the per-core kernel for `rpc` batch rows per core.

    loop_iters: timing-only mode — repeat the whole pass that many times
    inside a hardware For_i loop (same data each iteration).
    """
    nb = rpc // BB
    nc = bacc.Bacc("TRN2", target_bir_lowering=False, debug=False,
                   num_devices=N_CORES)

    # s / W1 arrive in bf16: mm1 runs at the same PE rate (1 cycle/row) but
    # the startup-critical DMA halves, which is what gates the first block
    sTd = nc.dram_tensor("sT", [S_DIM, rpc], bf16, kind="ExternalInput").ap()
    u2d = nc.dram_tensor("u2", [rpc, 2 * A_DIM], f32, kind="ExternalInput").ap()
    W1d = nc.dram_tensor("W1d", [S_DIM, H_DIM], bf16, kind="ExternalInput").ap()
    # W2/Wd arrive host-packed partition-major so one partition's data is a
    # single contiguous run (DMA chunk size drives HBM efficiency)
    W2d = nc.dram_tensor("W2d", [128, 8 * D_HEAD], f32r, kind="ExternalInput").ap()
    Wdd = nc.dram_tensor("Wdd", [128, 2 * A_DIM], f32r, kind="ExternalInput").ap()
    b1d = nc.dram_tensor("b1d", [128, 8], f32, kind="ExternalInput").ap()
    b2d = nc.dram_tensor("b2d", [128, 2], f32, kind="ExternalInput").ap()
    identd = nc.dram_tensor("identd", [128, 128], f32r,
                            kind="ExternalInput").ap()
    if has_db:
        dbd = nc.dram_tensor("dbd", [1, A_DIM], f32r, kind="ExternalInput").ap()
    # only the even elements of each output pair are shipped (odd = 1 - even),
    # as uint8 {0,1} — exact, since the fp32 output is exactly one-hot
    outd = nc.dram_tensor("out", [rpc, A_DIM], mybir.dt.uint8,
                          kind="ExternalOutput").ap()

    sTv = sTd.rearrange("(a p) b -> p a b", p=128)      # [128, 8, rpc]
    # u arrives host-permuted in row pairs: row = q*256 + p*2 + h, so each
    # partition line of a pair-load is 8KB contiguous
    u2v = u2d.rearrange("(q p h) m -> p q (h m)", p=128, h=2)
    # out leaves partition-major within each 2-block group: DRAM row
    # g*(128*x) + p*x + xx, so each group store writes x*512 contiguous
    # bytes per partition (host undoes the permutation)
    xgrp = 8 if nb % 2 == 0 else 4
    assert nb % 2 == 0 or nb == 1
    outv = outd.rearrange("(g p x) m -> p g x m", p=128, x=xgrp)

    with tile.TileContext(nc) as tc, ExitStack() as ctx:
        wp = ctx.enter_context(tc.tile_pool(name="weights", bufs=1))
        sT_pool = ctx.enter_context(tc.tile_pool(name="sTp", bufs=4))
        u_pool = ctx.enter_context(tc.tile_pool(name="up", bufs=4))
        p_pool = ctx.enter_context(tc.tile_pool(name="pp", bufs=2))
        q_pool = ctx.enter_context(tc.tile_pool(name="qp", bufs=6))
        gdn_pool = ctx.enter_context(tc.tile_pool(name="gdnp", bufs=10))
        hT_pool = ctx.enter_context(tc.tile_pool(name="hTp", bufs=10))
        awT_pool = ctx.enter_context(tc.tile_pool(name="awTp", bufs=5))
        out_pool = ctx.enter_context(tc.tile_pool(name="outp", bufs=3))
        # h and d tiles share one 6-slot tag so mm1 (block0's k-outer) and
        # mm3 time-share PSUM banks; awT holds the other 2 banks.
        hd_psp = ctx.enter_context(tc.tile_pool(name="hdps", bufs=6, space="PSUM"))
        a_psp = ctx.enter_context(tc.tile_pool(name="aps", bufs=2, space="PSUM"))

        W1v = W1d.rearrange("(a p) j -> p a j", p=128)
        if loop_iters is not None:
            loop_cm = tc.For_i(0, loop_iters, 1)
            loop_cm.__enter__()
        # sT is processed in 2-block groups so each DMA's partition line is
        # a 4KB contiguous run; group 0 is loaded chunk-by-chunk interleaved
        # with W1 so block0's k-outer matmuls track the DMA stream.
        ngrp = (nb + 1) // 2

        def g_cols(g):
            return min(2 * BB, rpc - g * 2 * BB)

        def sT_group_load(g, split):
            tiles = []
            for ka in range(2):
                sT_t = sT_pool.tile([128, 4, 2 * BB], bf16, name="sT_t")
                if not split:
                    nc.sync.dma_start(
                        sT_t[:, :, 0:g_cols(g)],
                        sTv[:, ka * 4:(ka + 1) * 4, ds(g * 2 * BB, g_cols(g))])
                tiles.append(sT_t)
            return tiles

        W1_sb = wp.tile([128, 8, H_DIM], bf16)
        g0_ts = sT_group_load(0, split=True)
        # stream block-0's needs first: per k, the W1 chunk + only block-0's
        # 512 sT columns (1092ns DMA vs 1278ns of 6-wide j-group PE work per
        # k, so the PE never starves); block-1's columns follow afterwards
        gc0 = g_cols(0)
        ca = min(BB, gc0)
        for k in range(8):
            if k == 0:
                # k0's sT chunk goes through the Pool SWDGE pipe, in parallel
                # with W1 k0 on HWDGE: the first matmul fires ~250ns earlier
                # and a freed HWDGE slot tightens the whole k1+ ladder
                nc.sync.dma_start(W1_sb[:, 0, 0:512], W1v[:, 0, 0:512])
                nc.gpsimd.dma_start(g0_ts[0][:, 0, 0:ca], sTv[:, 0, ds(0, ca)])
                nc.sync.dma_start(W1_sb[:, 0, 512:1024], W1v[:, 0, 512:1024])
            else:
                nc.sync.dma_start(W1_sb[:, k, :], W1v[:, k, :])
                nc.sync.dma_start(g0_ts[k // 4][:, k % 4, 0:ca],
                                  sTv[:, k, ds(0, ca)])
        b1_sb = wp.tile([128, 8], f32)
        nc.sync.dma_start(b1_sb[:], b1d[:])
        b2_sb = wp.tile([128, 2], f32)
        nc.sync.dma_start(b2_sb[:], b2d[:])
        W2_sb = wp.tile([128, 8 * D_HEAD], f32r)
        # split so mm2's first j-chunks don't wait on the whole 1MB load
        nc.sync.dma_start(W2_sb[:, 0:2 * D_HEAD], W2d[:, 0:2 * D_HEAD])
        nc.sync.dma_start(W2_sb[:, 2 * D_HEAD:], W2d[:, 2 * D_HEAD:])
        W2_v = W2_sb.rearrange("p (j d) -> p j d", j=8)
        Wd_sb = wp.tile([128, 2 * A_DIM], f32r)
        Wd_v = Wd_sb.rearrange("p (a m) -> p a m", a=2)
        ident_sb = wp.tile([128, 128], f32r)
        # block-1's sT columns and Wd interleave by need time: block-1 mm1
        # starts ~15us, the deferred block-0 mm3 needs Wd just after
        if gc0 > BB:
            for k in range(4):
                nc.sync.dma_start(g0_ts[k // 4][:, k % 4, BB:gc0],
                                  sTv[:, k, ds(BB, gc0 - BB)])
        nc.sync.dma_start(Wd_sb[:], Wdd[:])
        nc.sync.dma_start(ident_sb[:], identd[:])
        if gc0 > BB:
            for k in range(4, 8):
                nc.sync.dma_start(g0_ts[k // 4][:, k % 4, BB:gc0],
                                  sTv[:, k, ds(BB, gc0 - BB)])
        eps_sb = wp.tile([128, 1], f32)
        nc.vector.memset(eps_sb[:], EPS)
        if has_db:
            db_sb = wp.tile([1, A_DIM], f32r)
            nc.sync.dma_start(db_sb[:], dbd[:])
            ones_sb = wp.tile([1, 128], f32r)
            nc.vector.memset(ones_sb[:].bitcast(f32), 1.0)

        cur_sT = g0_ts
        next_sT = None
        o_t = None
        # mm3/compare/store of block b are deferred into block b+1's mm1
        # (after its j==0 column) so the mm2->awT->mm3 seam never stalls PE
        pending = None

        def u_load(b):
            # u row-pairs for block b (each an 8KB-line 1MB load)
            up_ts = []
            for q in range(2):
                u_t = u_pool.tile([128, 2, 2 * A_DIM], f32, name="u_t")
                nc.sync.dma_start(
                    u_t[:].rearrange("p h m -> p (h m)"),
                    u2v[:, b * 2 + q, :])
                up_ts.append(u_t)
            return up_ts

        cur_up = u_load(0)
        next_up = None

        def emit_mm3(awTs, gdns, ot, g_, bw_, tail=False):
            if not tail:
                for bs in range(4):
                    d_ps = hd_psp.tile([128, A_DIM], f32, name="d_ps",
                                       tag="ps")
                    for dt_ in range(2):
                        nc.tensor.matmul(d_ps[:], awTs[dt_][:, ts(bs, 128)],
                                         Wd_v[:, dt_, :], start=(dt_ == 0),
                                         stop=(dt_ == 1 and not has_db))
                    if has_db:
                        nc.tensor.matmul(d_ps[:], ones_sb[:], db_sb[:],
                                         start=False, stop=True)
                    nc.vector.tensor_tensor(ot[:, bw_ * 4 + bs, :], d_ps[:],
                                            gdns[bs][:], OP.is_ge)
                    # store each 128-row slice as soon as it's compared: the
                    # kernel tail then only waits on one 64KB store
                    nc.sync.dma_start(outv[:, g_, bw_ * 4 + bs, :],
                                      ot[:, bw_ * 4 + bs, :])
                return
            # tail layout: odd slices fold -gdn into PSUM up front (no awT
            # dependency — these matmuls fill the mm2->awT seam), then all
            # dt0 matmuls (gated only on awT0), then all dt1. Odd slices
            # compare via Sign on ACT (halving the DVE chain) and store via
            # the Pool SWDGE path so descriptor-gen runs in parallel with
            # HWDGE's.
            d_pss = []
            for bs in range(4):
                d_ps = hd_psp.tile([128, A_DIM], f32, name="d_ps", tag="ps")
                d_pss.append(d_ps)
                if bs % 2 == 1:
                    nc.tensor.matmul(d_ps[:], ident_sb[:], gdns[bs][:],
                                     start=True, stop=False)
            for dt_ in range(2):
                # odd (Sign/Pool-store) slices first in the dt1 round so the
                # longest store chain starts earliest
                for bs in ((1, 3, 0, 2) if dt_ == 1 else range(4)):
                    nc.tensor.matmul(d_pss[bs][:], awTs[dt_][:, ts(bs, 128)],
                                     Wd_v[:, dt_, :],
                                     start=(dt_ == 0 and bs % 2 == 0),
                                     stop=(dt_ == 1 and not has_db),
                                     skip_group_check=True)
                    if dt_ == 1 and has_db:
                        nc.tensor.matmul(d_pss[bs][:], ones_sb[:], db_sb[:],
                                         start=False, stop=True)
                    if dt_ == 1:
                        sl = ot[:, bw_ * 4 + bs, :]
                        if bs % 2 == 1:
                            nc.scalar.activation(sl, d_pss[bs][:], AFT.Sign,
                                                 bias=0.0, scale=1.0)
                        else:
                            nc.vector.tensor_tensor(sl, d_pss[bs][:],
                                                    gdns[bs][:], OP.is_ge)
                        # queue split chosen so the 625ns-per-DMA HWDGE chain
                        # and the 1038ns Pool SWDGE gen finish together
                        if bs == 0:
                            nc.gpsimd.dma_start(outv[:, g_, bw_ * 4 + bs, :],
                                                sl)
                        else:
                            nc.sync.dma_start(outv[:, g_, bw_ * 4 + bs, :],
                                              sl)

        for b0 in range(nb):
            g = b0 // 2
            bw = b0 % 2
            if bw == 0:
                o_t = out_pool.tile([128, 8, A_DIM], mybir.dt.uint8,
                                    name="o_t")
            else:
                # prefetch the NEXT 2-block sT group one block ahead — late
                # enough to keep the head DMA queue short, early enough that
                # the 4MB lands within one block period
                if g + 1 < ngrp:
                    next_sT = sT_group_load(g + 1, split=False)
            # u is prefetched one block ahead so the Lns (and the awT adds
            # queued behind them on ACT) never head-of-line block on u DMA
            up_ts = cur_up
            if b0 + 1 < nb:
                next_up = u_load(b0 + 1)
            u_ts = [up_ts[bs // 2][:, bs % 2, :] for bs in range(4)]

            def sT_at(k):
                return cur_sT[k // 4][:, k % 4, ds(bw * BB, BB)]

            # ---- gumbel: p = ln(u+eps); q = ln(-p+eps); gdn = q0-q1 ----
            # (for block 0 this is emitted after the matmuls: u arrives late
            # and the Lns must not block the relus in the ACT FIFO)
            gdn_ts = []
            q_ts = []

            def ln_part(bs):
                p_t = p_pool.tile([128, 2 * A_DIM], f32, name="p_t")
                nc.scalar.activation(p_t[:], u_ts[bs], AFT.Ln,
                                     bias=eps_sb[:], scale=1.0)
                q_t = q_pool.tile([128, 2 * A_DIM], f32, name="q_t")
                nc.scalar.activation(q_t[:], p_t[:], AFT.Ln,
                                     bias=eps_sb[:], scale=-1.0)
                q_ts.append(q_t)

            def gdn_part(bs):
                q_t = q_ts[bs]
                if b0 == nb - 1 and bs % 2 == 1:
                    # last block, odd slices: negated gdn in f32r, folded
                    # into PSUM by an identity matmul (fills the mm2->awT
                    # seam with PE work); the compare then runs as Sign on
                    # the otherwise-idle ACT engine
                    gdn_t = gdn_pool.tile([128, A_DIM], f32r, name="gdn_t",
                                          tag="gdnn", bufs=2)
                    nc.vector.tensor_tensor(gdn_t[:], q_t[:, 1::2],
                                            q_t[:, 0::2], OP.subtract)
                else:
                    gdn_t = gdn_pool.tile([128, A_DIM], f32, name="gdn_t")
                    nc.vector.tensor_tensor(gdn_t[:], q_t[:, 0::2],
                                            q_t[:, 1::2], OP.subtract)
                gdn_ts.append(gdn_t)

            def gumbel(bs):
                ln_part(bs)
                gdn_part(bs)

            if b0 > 0:
                # Lns go to the ACT queue now; the gdn DVE subs are emitted
                # late in the mm1 loop so they don't delay the relus there.
                for bs in range(4):
                    ln_part(bs)

            # ---- mm1 (+ mm2 interleaved one j-group behind for b0>0) ----
            hT_ts = []
            a_pss = [a_psp.tile([128, BB], f32, name="a_ps") for _ in range(2)]

            def mm2_partial(j):
                for dt_ in range(2):
                    nc.tensor.matmul(a_pss[dt_][:], W2_v[:, j, ts(dt_, 128)],
                                     hT_ts[j][:], start=(j == 0),
                                     stop=(j == 7), skip_group_check=True)

            if b0 == 0:
                # k-outer over j-groups of 6 then 2: the widest group keeps PE
                # ~90% busy while the W1/sT0 chunks stream in; first matmuls
                # need only W1 chunk 0 + the first sT quarter.
                for grp in (range(0, 6), range(6, 8)):
                    h_pss = [hd_psp.tile([128, BB], f32, name="h_ps",
                                         tag="ps") for _ in grp]
                    for k in range(8):
                        for gi, j in enumerate(grp):
                            nc.tensor.matmul(
                                h_pss[gi][:], W1_sb[:, k, ts(j, 128)],
                                sT_at(k), start=(k == 0), stop=(k == 7))
                    for gi, j in enumerate(grp):
                        hT_t = hT_pool.tile([128, BB], f32r, name="hT_t")
                        nc.vector.tensor_scalar(hT_t[:], h_pss[gi][:],
                                                b1_sb[:, j:j + 1], 0.0,
                                                OP.add, OP.max)
                        hT_ts.append(hT_t)
                for j in range(8):
                    mm2_partial(j)
            else:
                last = b0 == nb - 1
                jmax = 7 if last else 8
                for j in range(jmax):
                    h_ps = hd_psp.tile([128, BB], f32, name="h_ps", tag="ps")
                    for k in range(8):
                        nc.tensor.matmul(h_ps[:], W1_sb[:, k, ts(j, 128)],
                                         sT_at(k), start=(k == 0),
                                         stop=(k == 7))
                    hT_t = hT_pool.tile([128, BB], f32r, name="hT_t")
                    nc.vector.tensor_scalar(hT_t[:], h_ps[:],
                                            b1_sb[:, j:j + 1], 0.0,
                                            OP.add, OP.max)
                    hT_ts.append(hT_t)
                    if j == 0 and pending is not None:
                        emit_mm3(*pending)
                        pending = None
                    if last and 3 <= j <= 6:
                        # tail folds read the gdns right after mm2(7), so
                        # they must precede the j7 relus in the DVE queue
                        gdn_part(j - 3)
                    if j >= 2:
                        mm2_partial(j - 2)
                if not last:
                    # the gdn DVE subs go after all relus: they're only read
                    # by the deferred compares one block later, and keeping
                    # them off the relu->mm2 path avoids pinching the
                    # in-order DVE queue
                    for bs in range(4):
                        gdn_part(bs)
                    mm2_partial(6)
                    mm2_partial(7)
                else:
                    # ---- last block: j7 + the whole epilogue run in two
                    # 256-column halves, so half 0's mm3/compare/store chain
                    # overlaps half 1's matmuls and the final compare chain
                    # is one op deep per engine instead of two ----
                    hT7 = hT_pool.tile([128, BB], f32r, name="hT_t")
                    for h in range(2):
                        h_ps = hd_psp.tile([128, 256], f32, name="h_ps",
                                           tag="ps")
                        for k in range(8):
                            nc.tensor.matmul(
                                h_ps[:], W1_sb[:, k, ts(7, 128)],
                                cur_sT[k // 4][:, k % 4,
                                               ds(bw * BB + h * 256, 256)],
                                start=(k == 0), stop=(k == 7))
                        nc.vector.tensor_scalar(hT7[:, h * 256:h * 256 + 256],
                                                h_ps[:], b1_sb[:, 7:8], 0.0,
                                                OP.add, OP.max)
                        mm2_partial(5 + h)
                    hT_ts.append(hT7)
                    for h in range(2):
                        sl2 = slice(h * 256, h * 256 + 256)
                        for dt_ in range(2):
                            nc.tensor.matmul(a_pss[dt_][:, sl2],
                                             W2_v[:, 7, ts(dt_, 128)],
                                             hT7[:, sl2], start=False,
                                             stop=True, skip_group_check=True)
                    dps = []
                    for bs in range(4):
                        d_ps = hd_psp.tile([128, A_DIM], f32,
                                           name="d_ps", tag="ps")
                        if bs % 2 == 1:
                            nc.tensor.matmul(d_ps[:], ident_sb[:],
                                             gdn_ts[bs][:], start=True,
                                             stop=False)
                        dps.append(d_ps)
                    awThs = []
                    for h in range(2):
                        sl2 = slice(h * 256, h * 256 + 256)
                        awTh = []
                        for dt_ in range(2):
                            awT_t = awT_pool.tile([128, 256], f32r,
                                                  name="awT_t", tag="awh",
                                                  bufs=4)
                            if dt_ == 1:
                                nc.vector.tensor_scalar_add(
                                    awT_t[:], a_pss[dt_][:, sl2],
                                    b2_sb[:, dt_:dt_ + 1])
                            else:
                                nc.scalar.activation(
                                    awT_t[:], a_pss[dt_][:, sl2],
                                    AFT.Identity,
                                    bias=b2_sb[:, dt_:dt_ + 1], scale=1.0)
                            awTh.append(awT_t)
                        awThs.append(awTh)
                    for h in range(2):
                        for dt_ in range(2):
                            for i_ in range(2):
                                bs = 2 * h + i_
                                nc.tensor.matmul(
                                    dps[bs][:], awThs[h][dt_][:, ts(i_, 128)],
                                    Wd_v[:, dt_, :],
                                    start=(dt_ == 0 and i_ == 0),
                                    stop=(dt_ == 1 and not has_db),
                                    skip_group_check=True)
                                if dt_ == 1 and has_db:
                                    nc.tensor.matmul(dps[bs][:], ones_sb[:],
                                                     db_sb[:], start=False,
                                                     stop=True)
                        for i_ in range(2):
                            bs = 2 * h + i_
                            sl = o_t[:, bw * 4 + bs, :]
                            if i_ == 1:
                                nc.scalar.activation(sl, dps[bs][:], AFT.Sign,
                                                     bias=0.0, scale=1.0)
                            else:
                                nc.vector.tensor_tensor(sl, dps[bs][:],
                                                        gdn_ts[bs][:],
                                                        OP.is_ge)
                            # the very last slice takes the Pool SWDGE path
                            # (its gen runs while HWDGE drains the others)
                            if bs == 3:
                                nc.gpsimd.dma_start(
                                    outv[:, g, bw * 4 + bs, :], sl)
                            else:
                                nc.sync.dma_start(
                                    outv[:, g, bw * 4 + bs, :], sl)
                    cur_up = next_up
                    cur_sT = next_sT
                    continue

            last = b0 == nb - 1
            awT_ts = []
            for dt_ in range(2):
                awT_t = awT_pool.tile([128, BB], f32r, name="awT_t")
                if b0 <= 2 or (last and dt_ == 1):
                    # blocks 0-2: the ACT Ln pipeline is still draining the
                    # startup u backlog, so awT would head-of-line block
                    # behind u-gated Lns there; last block: split engines so
                    # both mm3 operands are ready ~one op after mm2
                    nc.vector.tensor_scalar_add(awT_t[:], a_pss[dt_][:],
                                                b2_sb[:, dt_:dt_ + 1])
                else:
                    # ACT can read PSUM and Identity shares the Ln table set,
                    # so these adds cost no DVE time and no table reload
                    nc.scalar.activation(awT_t[:], a_pss[dt_][:],
                                         AFT.Identity,
                                         bias=b2_sb[:, dt_:dt_ + 1],
                                         scale=1.0)
                awT_ts.append(awT_t)

            if b0 == 0:
                for bs in range(4):
                    gumbel(bs)

            if last:
                if pending is not None:
                    emit_mm3(*pending)
                    pending = None
                emit_mm3(awT_ts, gdn_ts, o_t, g, bw, tail=True)
            else:
                pending = (awT_ts, gdn_ts, o_t, g, bw)
            cur_up = next_up
            if (bw == 1) or last:
                cur_sT = next_sT

        if loop_iters is not None:
            loop_cm.__exit__(None, None, None)

    nc.compile()
    return nc


def kernel(s, u, W1, b1, W2, b2, head_W, head_b, _rpc=None):
    global LAST_EXEC_NS
    s = np.asarray(s, dtype=np.float32)
    u = np.asarray(u, dtype=np.float32)
    W1 = np.ascontiguousarray(np.asarray(W1, dtype=np.float32))
    W2 = np.ascontiguousarray(np.asarray(W2, dtype=np.float32))
    b1 = np.asarray(b1, dtype=np.float32)
    b2 = np.asarray(b2, dtype=np.float32)
    head_W = np.asarray(head_W, dtype=np.float32)
    head_b = np.asarray(head_b, dtype=np.float32)

    nrows = s.shape[0]
    rpc = _rpc if _rpc is not None else nrows // N_CORES
    assert nrows == rpc * N_CORES and rpc % BB == 0

    sT = s.T.astype(ml_dtypes.bfloat16)                 # [S_DIM, nrows]
    u2 = u.reshape(nrows, 2 * A_DIM)
    # permute u rows to (q, p, h) pair-major per core shard (done per core
    # below), pack W2/Wd partition-major
    Wd = np.ascontiguousarray((head_W[:, :, 0] - head_W[:, :, 1]).T)
    W2h = np.ascontiguousarray(
        W2.reshape(8, 128, D_HEAD).transpose(1, 0, 2)).reshape(128, 8 * D_HEAD)
    Wdh = np.ascontiguousarray(
        Wd.reshape(2, 128, A_DIM).transpose(1, 0, 2)).reshape(128, 2 * A_DIM)
    db = np.ascontiguousarray(head_b[:, 0] - head_b[:, 1]).reshape(1, A_DIM)
    has_db = bool(np.any(db))
    b1c = np.ascontiguousarray(b1.reshape(8, 128).T)
    b2c = np.ascontiguousarray(b2.reshape(2, 128).T)

    key = (rpc, has_db)
    if key not in _cache:
        _cache[key] = _build(rpc, has_db)
    nc = _cache[key]

    nq = rpc // 256
    in_maps = []
    for c in range(N_CORES):
        uc = u2[c * rpc:(c + 1) * rpc]
        up = np.ascontiguousarray(
            uc.reshape(nq, 2, 128, 2 * A_DIM).transpose(0, 2, 1, 3)
        ).reshape(rpc, 2 * A_DIM)
        m = {
            "sT": np.ascontiguousarray(sT[:, c * rpc:(c + 1) * rpc]),
            "u2": up,
            "W1d": W1.astype(ml_dtypes.bfloat16),
            "W2d": W2h, "Wdd": Wdh, "b1d": b1c, "b2d": b2c,
            "identd": np.eye(128, dtype=np.float32),
        }
        if has_db:
            m["dbd"] = db
        in_maps.append(m)

    res = bass_utils.run_bass_kernel_spmd(nc, in_maps,
                                          core_ids=list(range(N_CORES)))
    LAST_EXEC_NS = res.exec_time_ns
    nb = rpc // BB
    xgrp = 8 if nb % 2 == 0 else 4
    shards = []
    for c in range(N_CORES):
        e = res.results[c]["out"]                        # [rpc, A_DIM] uint8
        # undo the (g, p, x) store permutation back to batch order
        e = e.reshape(rpc // (128 * xgrp), 128, xgrp, A_DIM)
        shards.append(e.transpose(0, 2, 1, 3).reshape(rpc, A_DIM))
    evens = np.concatenate(shards, axis=0)               # [nrows, A_DIM]
    out = np.empty((nrows, 2 * A_DIM), dtype=np.float32)
    ef = evens.astype(np.float32)
    out[:, 0::2] = ef
    out[:, 1::2] = 1.0 - ef
    return out



# revision 52
# speedup vs baseline: 1.1189x; 1.0017x over previous
"""Trainium2 Bass kernel for nn_MultiDiscretePolicy.

Math:
  h   = relu(s @ W1 + b1)                         [B, 1024]
  aw  = h @ W2 + b2                               [B, 256]
  d   = aw @ Wd + db    (Wd = head_W[...,0]-head_W[...,1] transposed)
  out pair h: even = 1.0 if (logit0+g0) >= (logit1+g1) else 0.0, odd = 1-even
The reference's y + stop_grad(y_hard - y) is exactly one-hot in fp32, and
argmax(softmax(z)) == argmax(z), so the output reduces to the sign test
  even = (d >= gdn),  gdn = q0 - q1,  q_k = log(-log(u_k + EPS) + EPS)
(two Ln passes on ACT; Ln/Identity/Sign share one activation table set).

Sharding: pure data parallel over the batch dim across 8 cores.  Matmuls keep
features on PSUM partitions / batch on the moving free dim, so the only
transpose needed (s^T) is done on host.

The kernel is PE-flop-bound (~151us of matmul at 1 cycle/row), so the layout
is built around keeping the tensor engine saturated (94.7% occupancy):
  - mm1 runs in bf16 (same PE rate as f32r, half the DMA): the startup
    W1+sT stream halves, which is what gated block 0.  Costs ~960 flipped
    sign-test pairs out of 16.8M (~1.1e-2 rel err, budget is 2e-2).
  - mm2/mm3 stay f32r (12-bit mantissa products, fp32 accumulate).
  - startup k0's sT chunk goes through the Pool SWDGE pipe in parallel with
    W1 on HWDGE; with the freed HWDGE slot the W1/sT k-ladder feeds the
    block-0 k-outer matmuls with no PE stalls from 3.7us on.
  - mm3/compare/store of block b are deferred into block b+1's mm1 so the
    mm2->awT(ACT Identity, PSUM-readable)->mm3 seam never stalls PE; u is
    prefetched one block ahead so the ACT Ln queue never head-of-line
    blocks the awT adds; gdn DVE subs sit after the relus they would pinch.
  - the last block's j7/mm2/mm3 run in two 256-column halves; odd slices
    fold -gdn into PSUM via an identity matmul (filling the awT seam) and
    compare via Sign on the idle ACT engine (saturates to exact {0,1} in
    uint8), even slices compare is_ge on DVE, and each half stores as one
    DMA: the post-matmul tail is ~3.9us (Sign + store + DMA-sem + drain),
    all per-DMA constants.
"""
from contextlib import ExitStack

import ml_dtypes
import numpy as np

import concourse.bass as bass
import concourse.mybir as mybir
import concourse.tile as tile
from concourse import bacc
from concourse import bass_utils
from concourse.bass import ts, ds

N_CORES = 8
B, S_DIM, H_DIM, A_DIM = 32768, 1024, 1024, 512
D_HEAD = A_DIM // 2
EPS = 1e-20
BB = 512           # batch columns per block (one PSUM bank of fp32)
f32 = mybir.dt.float32
f32r = mybir.dt.float32r
bf16 = mybir.dt.bfloat16
AFT = mybir.ActivationFunctionType
OP = mybir.AluOpType

LAST_EXEC_NS = None

_cache: dict = {}


def _build(rpc: int, has_db: bool, loop_iters: int | None = None):
    """Build the per-core kernel for `rpc` batch rows per core.

    loop_iters: timing-only mode — repeat the whole pass that many times
    inside a hardware For_i loop (same data each iteration).
    """
    nb = rpc // BB
    nc = bacc.Bacc("TRN2", target_bir_lowering=False, debug=False,
                   num_devices=N_CORES)

    # s / W1 arrive in bf16: mm1 runs at the same PE rate (1 cycle/row) but
    # the startup-critical DMA halves, which is what gates the first block
    sTd = nc.dram_tensor("sT", [S_DIM, rpc], bf16, kind="ExternalInput").ap()
    u2d = nc.dram_tensor("u2", [rpc, 2 * A_DIM], f32, kind="ExternalInput").ap()
    W1d = nc.dram_tensor("W1d", [S_DIM, H_DIM], bf16, kind="ExternalInput").ap()
    # W2/Wd arrive host-packed partition-major so one partition's data is a
    # single contiguous run (DMA chunk size drives HBM efficiency)
    W2d = nc.dram_tensor("W2d", [128, 8 * D_HEAD], f32r, kind="ExternalInput").ap()
    Wdd = nc.dram_tensor("Wdd", [128, 2 * A_DIM], f32r, kind="ExternalInput").ap()
    b1d = nc.dram_tensor("b1d", [128, 8], f32, kind="ExternalInput").ap()
    b2d = nc.dram_tensor("b2d", [128, 2], f32, kind="ExternalInput").ap()
    identd = nc.dram_tensor("identd", [128, 128], f32r,
                            kind="ExternalInput").ap()
    if has_db:
        dbd = nc.dram_tensor("dbd", [1, A_DIM], f32r, kind="ExternalInput").ap()
    # only the even elements of each output pair are shipped (odd = 1 - even),
    # as uint8 {0,1} — exact, since the fp32 output is exactly one-hot
    outd = nc.dram_tensor("out", [rpc, A_DIM], mybir.dt.uint8,
                          kind="ExternalOutput").ap()

    sTv = sTd.rearrange("(a p) b -> p a b", p=128)      # [128, 8, rpc]
    # u arrives host-permuted in row pairs: row = q*256 + p*2 + h, so each
    # partition line of a pair-load is 8KB contiguous
    u2v = u2d.rearrange("(q p h) m -> p q (h m)", p=128, h=2)
    # out leaves partition-major within each 2-block group: DRAM row
    # g*(128*x) + p*x + xx, so each group store writes x*512 contiguous
    # bytes per partition (host undoes the permutation)
    xgrp = 8 if nb % 2 == 0 else 4
    assert nb % 2 == 0 or nb == 1
    outv = outd.rearrange("(g p x) m -> p g x m", p=128, x=xgrp)

    with tile.TileContext(nc) as tc, ExitStack() as ctx:
        wp = ctx.enter_context(tc.tile_pool(name="weights", bufs=1))
        sT_pool = ctx.enter_context(tc.tile_pool(name="sTp", bufs=4))
        u_pool = ctx.enter_context(tc.tile_pool(name="up", bufs=4))
        p_pool = ctx.enter_context(tc.tile_pool(name="pp", bufs=2))
        q_pool = ctx.enter_context(tc.tile_pool(name="qp", bufs=6))
        gdn_pool = ctx.enter_context(tc.tile_pool(name="gdnp", bufs=10))
        hT_pool = ctx.enter_context(tc.tile_pool(name="hTp", bufs=10))
        awT_pool = ctx.enter_context(tc.tile_pool(name="awTp", bufs=5))
        out_pool = ctx.enter_context(tc.tile_pool(name="outp", bufs=3))
        # h and d tiles share one 6-slot tag so mm1 (block0's k-outer) and
        # mm3 time-share PSUM banks; awT holds the other 2 banks.
        hd_psp = ctx.enter_context(tc.tile_pool(name="hdps", bufs=6, space="PSUM"))
        a_psp = ctx.enter_context(tc.tile_pool(name="aps", bufs=2, space="PSUM"))

        W1v = W1d.rearrange("(a p) j -> p a j", p=128)
        if loop_iters is not None:
            loop_cm = tc.For_i(0, loop_iters, 1)
            loop_cm.__enter__()
        # sT is processed in 2-block groups so each DMA's partition line is
        # a 4KB contiguous run; group 0 is loaded chunk-by-chunk interleaved
        # with W1 so block0's k-outer matmuls track the DMA stream.
        ngrp = (nb + 1) // 2

        def g_cols(g):
            return min(2 * BB, rpc - g * 2 * BB)

        def sT_group_load(g, split):
            tiles = []
            for ka in range(2):
                sT_t = sT_pool.tile([128, 4, 2 * BB], bf16, name="sT_t")
                if not split:
                    nc.sync.dma_start(
                        sT_t[:, :, 0:g_cols(g)],
                        sTv[:, ka * 4:(ka + 1) * 4, ds(g * 2 * BB, g_cols(g))])
                tiles.append(sT_t)
            return tiles

        W1_sb = wp.tile([128, 8, H_DIM], bf16)
        g0_ts = sT_group_load(0, split=True)
        # stream block-0's needs first: per k, the W1 chunk + only block-0's
        # 512 sT columns (1092ns DMA vs 1278ns of 6-wide j-group PE work per
        # k, so the PE never starves); block-1's columns follow afterwards
        gc0 = g_cols(0)
        ca = min(BB, gc0)
        for k in range(8):
            if k == 0:
                # k0's sT chunk goes through the Pool SWDGE pipe, in parallel
                # with W1 k0 on HWDGE: the first matmul fires ~250ns earlier
                # and a freed HWDGE slot tightens the whole k1+ ladder
                nc.sync.dma_start(W1_sb[:, 0, 0:512], W1v[:, 0, 0:512])
                nc.gpsimd.dma_start(g0_ts[0][:, 0, 0:ca], sTv[:, 0, ds(0, ca)])
                nc.sync.dma_start(W1_sb[:, 0, 512:1024], W1v[:, 0, 512:1024])
            else:
                nc.sync.dma_start(W1_sb[:, k, :], W1v[:, k, :])
                nc.sync.dma_start(g0_ts[k // 4][:, k % 4, 0:ca],
                                  sTv[:, k, ds(0, ca)])
        b1_sb = wp.tile([128, 8], f32)
        nc.sync.dma_start(b1_sb[:], b1d[:])
        b2_sb = wp.tile([128, 2], f32)
        nc.sync.dma_start(b2_sb[:], b2d[:])
        W2_sb = wp.tile([128, 8 * D_HEAD], f32r)
        # split so mm2's first j-chunks don't wait on the whole 1MB load
        nc.sync.dma_start(W2_sb[:, 0:2 * D_HEAD], W2d[:, 0:2 * D_HEAD])
        nc.sync.dma_start(W2_sb[:, 2 * D_HEAD:], W2d[:, 2 * D_HEAD:])
        W2_v = W2_sb.rearrange("p (j d) -> p j d", j=8)
        Wd_sb = wp.tile([128, 2 * A_DIM], f32r)
        Wd_v = Wd_sb.rearrange("p (a m) -> p a m", a=2)
        ident_sb = wp.tile([128, 128], f32r)
        # block-1's sT columns and Wd interleave by need time: block-1 mm1
        # starts ~15us, the deferred block-0 mm3 needs Wd just after
        if gc0 > BB:
            for k in range(4):
                nc.sync.dma_start(g0_ts[k // 4][:, k % 4, BB:gc0],
                                  sTv[:, k, ds(BB, gc0 - BB)])
        nc.sync.dma_start(Wd_sb[:], Wdd[:])
        nc.sync.dma_start(ident_sb[:], identd[:])
        if gc0 > BB:
            for k in range(4, 8):
                nc.sync.dma_start(g0_ts[k // 4][:, k % 4, BB:gc0],
                                  sTv[:, k, ds(BB, gc0 - BB)])
        eps_sb = wp.tile([128, 1], f32)
        nc.vector.memset(eps_sb[:], EPS)
        if has_db:
            db_sb = wp.tile([1, A_DIM], f32r)
            nc.sync.dma_start(db_sb[:], dbd[:])
            ones_sb = wp.tile([1, 128], f32r)
            nc.vector.memset(ones_sb[:].bitcast(f32), 1.0)

        cur_sT = g0_ts
        next_sT = None
        o_t = None
        # mm3/compare/store of block b are deferred into block b+1's mm1
        # (after its j==0 column) so the mm2->awT->mm3 seam never stalls PE
        pending = None

        def u_load(b):
            # u row-pairs for block b (each an 8KB-line 1MB load)
            up_ts = []
            for q in range(2):
                u_t = u_pool.tile([128, 2, 2 * A_DIM], f32, name="u_t")
                nc.sync.dma_start(
                    u_t[:].rearrange("p h m -> p (h m)"),
                    u2v[:, b * 2 + q, :])
                up_ts.append(u_t)
            return up_ts

        cur_up = u_load(0)
        next_up = None

        def emit_mm3(awTs, gdns, ot, g_, bw_, tail=False):
            if not tail:
                for bs in range(4):
                    d_ps = hd_psp.tile([128, A_DIM], f32, name="d_ps",
                                       tag="ps")
                    for dt_ in range(2):
                        nc.tensor.matmul(d_ps[:], awTs[dt_][:, ts(bs, 128)],
                                         Wd_v[:, dt_, :], start=(dt_ == 0),
                                         stop=(dt_ == 1 and not has_db))
                    if has_db:
                        nc.tensor.matmul(d_ps[:], ones_sb[:], db_sb[:],
                                         start=False, stop=True)
                    nc.vector.tensor_tensor(ot[:, bw_ * 4 + bs, :], d_ps[:],
                                            gdns[bs][:], OP.is_ge)
                    # store each 128-row slice as soon as it's compared: the
                    # kernel tail then only waits on one 64KB store
                    nc.sync.dma_start(outv[:, g_, bw_ * 4 + bs, :],
                                      ot[:, bw_ * 4 + bs, :])
                return
            # tail layout: odd slices fold -gdn into PSUM up front (no awT
            # dependency — these matmuls fill the mm2->awT seam), then all
            # dt0 matmuls (gated only on awT0), then all dt1. Odd slices
            # compare via Sign on ACT (halving the DVE chain) and store via
            # the Pool SWDGE path so descriptor-gen runs in parallel with
            # HWDGE's.
            d_pss = []
            for bs in range(4):
                d_ps = hd_psp.tile([128, A_DIM], f32, name="d_ps", tag="ps")
                d_pss.append(d_ps)
                if bs % 2 == 1:
                    nc.tensor.matmul(d_ps[:], ident_sb[:], gdns[bs][:],
                                     start=True, stop=False)
            for dt_ in range(2):
                # odd (Sign/Pool-store) slices first in the dt1 round so the
                # longest store chain starts earliest
                for bs in ((1, 3, 0, 2) if dt_ == 1 else range(4)):
                    nc.tensor.matmul(d_pss[bs][:], awTs[dt_][:, ts(bs, 128)],
                                     Wd_v[:, dt_, :],
                                     start=(dt_ == 0 and bs % 2 == 0),
                                     stop=(dt_ == 1 and not has_db),
                                     skip_group_check=True)
                    if dt_ == 1 and has_db:
                        nc.tensor.matmul(d_pss[bs][:], ones_sb[:], db_sb[:],
                                         start=False, stop=True)
                    if dt_ == 1:
                        sl = ot[:, bw_ * 4 + bs, :]
                        if bs % 2 == 1:
                            nc.scalar.activation(sl, d_pss[bs][:], AFT.Sign,
                                                 bias=0.0, scale=1.0)
                        else:
                            nc.vector.tensor_tensor(sl, d_pss[bs][:],
                                                    gdns[bs][:], OP.is_ge)
                        # queue split chosen so the 625ns-per-DMA HWDGE chain
                        # and the 1038ns Pool SWDGE gen finish together
                        if bs == 0:
                            nc.gpsimd.dma_start(outv[:, g_, bw_ * 4 + bs, :],
                                                sl)
                        else:
                            nc.sync.dma_start(outv[:, g_, bw_ * 4 + bs, :],
                                              sl)

        for b0 in range(nb):
            g = b0 // 2
            bw = b0 % 2
            if bw == 0:
                o_t = out_pool.tile([128, 8, A_DIM], mybir.dt.uint8,
                                    name="o_t")
            else:
                # prefetch the NEXT 2-block sT group one block ahead — late
                # enough to keep the head DMA queue short, early enough that
                # the 4MB lands within one block period
                if g + 1 < ngrp:
                    next_sT = sT_group_load(g + 1, split=False)
            # u is prefetched one block ahead so the Lns (and the awT adds
            # queued behind them on ACT) never head-of-line block on u DMA
            up_ts = cur_up
            if b0 + 1 < nb:
                next_up = u_load(b0 + 1)
            u_ts = [up_ts[bs // 2][:, bs % 2, :] for bs in range(4)]

            def sT_at(k):
                return cur_sT[k // 4][:, k % 4, ds(bw * BB, BB)]

            # ---- gumbel: p = ln(u+eps); q = ln(-p+eps); gdn = q0-q1 ----
            # (for block 0 this is emitted after the matmuls: u arrives late
            # and the Lns must not block the relus in the ACT FIFO)
            gdn_ts = []
            q_ts = []

            def ln_part(bs):
                p_t = p_pool.tile([128, 2 * A_DIM], f32, name="p_t")
                nc.scalar.activation(p_t[:], u_ts[bs], AFT.Ln,
                                     bias=eps_sb[:], scale=1.0)
                q_t = q_pool.tile([128, 2 * A_DIM], f32, name="q_t")
                nc.scalar.activation(q_t[:], p_t[:], AFT.Ln,
                                     bias=eps_sb[:], scale=-1.0)
                q_ts.append(q_t)

            def gdn_part(bs):
                q_t = q_ts[bs]
                if b0 == nb - 1 and bs % 2 == 1:
                    # last block, odd slices: negated gdn in f32r, folded
                    # into PSUM by an identity matmul (fills the mm2->awT
                    # seam with PE work); the compare then runs as Sign on
                    # the otherwise-idle ACT engine
                    gdn_t = gdn_pool.tile([128, A_DIM], f32r, name="gdn_t",
                                          tag="gdnn", bufs=2)
                    nc.vector.tensor_tensor(gdn_t[:], q_t[:, 1::2],
                                            q_t[:, 0::2], OP.subtract)
                else:
                    gdn_t = gdn_pool.tile([128, A_DIM], f32, name="gdn_t")
                    nc.vector.tensor_tensor(gdn_t[:], q_t[:, 0::2],
                                            q_t[:, 1::2], OP.subtract)
                gdn_ts.append(gdn_t)

            def gumbel(bs):
                ln_part(bs)
                gdn_part(bs)

            if b0 > 0:
                # Lns go to the ACT queue now; the gdn DVE subs are emitted
                # late in the mm1 loop so they don't delay the relus there.
                for bs in range(4):
                    ln_part(bs)

            # ---- mm1 (+ mm2 interleaved one j-group behind for b0>0) ----
            hT_ts = []
            a_pss = [a_psp.tile([128, BB], f32, name="a_ps") for _ in range(2)]

            def mm2_partial(j):
                for dt_ in range(2):
                    nc.tensor.matmul(a_pss[dt_][:], W2_v[:, j, ts(dt_, 128)],
                                     hT_ts[j][:], start=(j == 0),
                                     stop=(j == 7), skip_group_check=True)

            if b0 == 0:
                # k-outer over j-groups of 6 then 2: the widest group keeps PE
                # ~90% busy while the W1/sT0 chunks stream in; first matmuls
                # need only W1 chunk 0 + the first sT quarter.
                for grp in (range(0, 6), range(6, 8)):
                    h_pss = [hd_psp.tile([128, BB], f32, name="h_ps",
                                         tag="ps") for _ in grp]
                    for k in range(8):
                        for gi, j in enumerate(grp):
                            nc.tensor.matmul(
                                h_pss[gi][:], W1_sb[:, k, ts(j, 128)],
                                sT_at(k), start=(k == 0), stop=(k == 7))
                    for gi, j in enumerate(grp):
                        hT_t = hT_pool.tile([128, BB], f32r, name="hT_t")
                        nc.vector.tensor_scalar(hT_t[:], h_pss[gi][:],
                                                b1_sb[:, j:j + 1], 0.0,
                                                OP.add, OP.max)
                        hT_ts.append(hT_t)
                for j in range(8):
                    mm2_partial(j)
            else:
                last = b0 == nb - 1
                jmax = 7 if last else 8
                for j in range(jmax):
                    h_ps = hd_psp.tile([128, BB], f32, name="h_ps", tag="ps")
                    for k in range(8):
                        nc.tensor.matmul(h_ps[:], W1_sb[:, k, ts(j, 128)],
                                         sT_at(k), start=(k == 0),
                                         stop=(k == 7))
                    hT_t = hT_pool.tile([128, BB], f32r, name="hT_t")
                    nc.vector.tensor_scalar(hT_t[:], h_ps[:],
                                            b1_sb[:, j:j + 1], 0.0,
                                            OP.add, OP.max)
                    hT_ts.append(hT_t)
                    if j == 0 and pending is not None:
                        emit_mm3(*pending)
                        pending = None
                    if last and 3 <= j <= 6:
                        # tail folds read the gdns right after mm2(7), so
                        # they must precede the j7 relus in the DVE queue
                        gdn_part(j - 3)
                    if j >= 2:
                        mm2_partial(j - 2)
                if not last:
                    # the gdn DVE subs go after all relus: they're only read
                    # by the deferred compares one block later, and keeping
                    # them off the relu->mm2 path avoids pinching the
                    # in-order DVE queue
                    for bs in range(4):
                        gdn_part(bs)
                    mm2_partial(6)
                    mm2_partial(7)
                else:
                    # ---- last block: j7 + the whole epilogue run in two
                    # 256-column halves, so half 0's mm3/compare/store chain
                    # overlaps half 1's matmuls and the final compare chain
                    # is one op deep per engine instead of two ----
                    hT7 = hT_pool.tile([128, BB], f32r, name="hT_t")
                    for h in range(2):
                        h_ps = hd_psp.tile([128, 256], f32, name="h_ps",
                                           tag="ps")
                        for k in range(8):
                            nc.tensor.matmul(
                                h_ps[:], W1_sb[:, k, ts(7, 128)],
                                cur_sT[k // 4][:, k % 4,
                                               ds(bw * BB + h * 256, 256)],
                                start=(k == 0), stop=(k == 7))
                        nc.vector.tensor_scalar(hT7[:, h * 256:h * 256 + 256],
                                                h_ps[:], b1_sb[:, 7:8], 0.0,
                                                OP.add, OP.max)
                        mm2_partial(5 + h)
                    hT_ts.append(hT7)
                    for h in range(2):
                        sl2 = slice(h * 256, h * 256 + 256)
                        for dt_ in range(2):
                            nc.tensor.matmul(a_pss[dt_][:, sl2],
                                             W2_v[:, 7, ts(dt_, 128)],
                                             hT7[:, sl2], start=False,
                                             stop=True, skip_group_check=True)
                    dps = []
                    for bs in range(4):
                        d_ps = hd_psp.tile([128, A_DIM], f32,
                                           name="d_ps", tag="ps")
                        if bs % 2 == 1:
                            nc.tensor.matmul(d_ps[:], ident_sb[:],
                                             gdn_ts[bs][:], start=True,
                                             stop=False)
                        dps.append(d_ps)
                    awThs = []
                    for h in range(2):
                        sl2 = slice(h * 256, h * 256 + 256)
                        awTh = []
                        for dt_ in range(2):
                            awT_t = awT_pool.tile([128, 256], f32r,
                                                  name="awT_t", tag="awh",
                                                  bufs=4)
                            if dt_ == 1:
                                nc.vector.tensor_scalar_add(
                                    awT_t[:], a_pss[dt_][:, sl2],
                                    b2_sb[:, dt_:dt_ + 1])
                            else:
                                nc.scalar.activation(
                                    awT_t[:], a_pss[dt_][:, sl2],
                                    AFT.Identity,
                                    bias=b2_sb[:, dt_:dt_ + 1], scale=1.0)
                            awTh.append(awT_t)
                        awThs.append(awTh)
                    for h in range(2):
                        for dt_ in range(2):
                            # dt1 finishes on the odd (Sign/ACT) slice: its
                            # 612ns ACT compare on an idle engine is the
                            # shortest possible final dependency chain
                            for i_ in ((0, 1) if dt_ == 1 else (1, 0)):
                                bs = 2 * h + i_
                                nc.tensor.matmul(
                                    dps[bs][:], awThs[h][dt_][:, ts(i_, 128)],
                                    Wd_v[:, dt_, :],
                                    start=(dt_ == 0 and i_ == 0),
                                    stop=(dt_ == 1 and not has_db),
                                    skip_group_check=True)
                                if dt_ == 1 and has_db:
                                    nc.tensor.matmul(dps[bs][:], ones_sb[:],
                                                     db_sb[:], start=False,
                                                     stop=True)
                        for i_ in range(2):
                            bs = 2 * h + i_
                            sl = o_t[:, bw * 4 + bs, :]
                            if i_ == 1:
                                nc.scalar.activation(sl, dps[bs][:], AFT.Sign,
                                                     bias=0.0, scale=1.0)
                            else:
                                nc.vector.tensor_tensor(sl, dps[bs][:],
                                                        gdn_ts[bs][:],
                                                        OP.is_ge)
                        # both slices of the half are adjacent in DRAM:
                        # one store per half keeps the final HWDGE queue
                        # two-deep instead of four-deep
                        x0 = bw * 4 + 2 * h
                        nc.sync.dma_start(outv[:, g, x0:x0 + 2, :],
                                          o_t[:, x0:x0 + 2, :])
                    cur_up = next_up
                    cur_sT = next_sT
                    continue

            last = b0 == nb - 1
            awT_ts = []
            for dt_ in range(2):
                awT_t = awT_pool.tile([128, BB], f32r, name="awT_t")
                if b0 <= 2 or (last and dt_ == 1):
                    # blocks 0-2: the ACT Ln pipeline is still draining the
                    # startup u backlog, so awT would head-of-line block
                    # behind u-gated Lns there; last block: split engines so
                    # both mm3 operands are ready ~one op after mm2
                    nc.vector.tensor_scalar_add(awT_t[:], a_pss[dt_][:],
                                                b2_sb[:, dt_:dt_ + 1])
                else:
                    # ACT can read PSUM and Identity shares the Ln table set,
                    # so these adds cost no DVE time and no table reload
                    nc.scalar.activation(awT_t[:], a_pss[dt_][:],
                                         AFT.Identity,
                                         bias=b2_sb[:, dt_:dt_ + 1],
                                         scale=1.0)
                awT_ts.append(awT_t)

            if b0 == 0:
                for bs in range(4):
                    gumbel(bs)

            if last:
                if pending is not None:
                    emit_mm3(*pending)
                    pending = None
                emit_mm3(awT_ts, gdn_ts, o_t, g, bw, tail=True)
            else:
                pending = (awT_ts, gdn_ts, o_t, g, bw)
            cur_up = next_up
            if (bw == 1) or last:
                cur_sT = next_sT

        if loop_iters is not None:
            loop_cm.__exit__(None, None, None)

    nc.compile()
    return nc


def kernel(s, u, W1, b1, W2, b2, head_W, head_b, _rpc=None):
    global LAST_EXEC_NS
    s = np.asarray(s, dtype=np.float32)
    u = np.asarray(u, dtype=np.float32)
    W1 = np.ascontiguousarray(np.asarray(W1, dtype=np.float32))
    W2 = np.ascontiguousarray(np.asarray(W2, dtype=np.float32))
    b1 = np.asarray(b1, dtype=np.float32)
    b2 = np.asarray(b2, dtype=np.float32)
    head_W = np.asarray(head_W, dtype=np.float32)
    head_b = np.asarray(head_b, dtype=np.float32)

    nrows = s.shape[0]
    rpc = _rpc if _rpc is not None else nrows // N_CORES
    assert nrows == rpc * N_CORES and rpc % BB == 0

    sT = s.T.astype(ml_dtypes.bfloat16)                 # [S_DIM, nrows]
    u2 = u.reshape(nrows, 2 * A_DIM)
    # permute u rows to (q, p, h) pair-major per core shard (done per core
    # below), pack W2/Wd partition-major
    Wd = np.ascontiguousarray((head_W[:, :, 0] - head_W[:, :, 1]).T)
    W2h = np.ascontiguousarray(
        W2.reshape(8, 128, D_HEAD).transpose(1, 0, 2)).reshape(128, 8 * D_HEAD)
    Wdh = np.ascontiguousarray(
        Wd.reshape(2, 128, A_DIM).transpose(1, 0, 2)).reshape(128, 2 * A_DIM)
    db = np.ascontiguousarray(head_b[:, 0] - head_b[:, 1]).reshape(1, A_DIM)
    has_db = bool(np.any(db))
    b1c = np.ascontiguousarray(b1.reshape(8, 128).T)
    b2c = np.ascontiguousarray(b2.reshape(2, 128).T)

    key = (rpc, has_db)
    if key not in _cache:
        _cache[key] = _build(rpc, has_db)
    nc = _cache[key]

    nq = rpc // 256
    in_maps = []
    for c in range(N_CORES):
        uc = u2[c * rpc:(c + 1) * rpc]
        up = np.ascontiguousarray(
            uc.reshape(nq, 2, 128, 2 * A_DIM).transpose(0, 2, 1, 3)
        ).reshape(rpc, 2 * A_DIM)
        m = {
            "sT": np.ascontiguousarray(sT[:, c * rpc:(c + 1) * rpc]),
            "u2": up,
            "W1d": W1.astype(ml_dtypes.bfloat16),
            "W2d": W2h, "Wdd": Wdh, "b1d": b1c, "b2d": b2c,
            "identd": np.eye(128, dtype=np.float32),
        }
        if has_db:
            m["dbd"] = db
        in_maps.append(m)

    res = bass_utils.run_bass_kernel_spmd(nc, in_maps,
                                          core_ids=list(range(N_CORES)))
    LAST_EXEC_NS = res.exec_time_ns
    nb = rpc // BB
    xgrp = 8 if nb % 2 == 0 else 4
    shards = []
    for c in range(N_CORES):
        e = res.results[c]["out"]                        # [rpc, A_DIM] uint8
        # undo the (g, p, x) store permutation back to batch order
        e = e.reshape(rpc // (128 * xgrp), 128, xgrp, A_DIM)
        shards.append(e.transpose(0, 2, 1, 3).reshape(rpc, A_DIM))
    evens = np.concatenate(shards, axis=0)               # [nrows, A_DIM]
    out = np.empty((nrows, 2 * A_DIM), dtype=np.float32)
    ef = evens.astype(np.float32)
    out[:, 0::2] = ef
    out[:, 1::2] = 1.0 - ef
    return out

